# revision 82
# baseline (speedup 1.0000x reference)
"""Trainium2 Bass kernel for nn_AutoregressiveArithmeticTransformer.

6-layer dense transformer: B=16, T=512, E=512, NH=8 heads x HS=64, FF=2048,
V=16, causal attention, pre-LN, learned abacus embedding, logits / 0.8.

Strategy: data-parallel over batch across 8 NeuronCores (2 sequences per
core, no collectives). Activations live feature-major in SBUF
([E-partitions, tokens]); weights are streamed per-layer in bf16; all
matmuls run in bf16 with fp32 PSUM accumulation; the residual stream stays
fp32. LayerNorm statistics are computed with ones-matmuls on the PE;
attention scores are computed transposed ([tk, tq]) so the softmax
denominator is also a ones-matmul; V is produced token-major directly so
no transposes are ever needed.

All ops are token-tile (512) granular so the two sequences per core form
independent dependency streams the Tile scheduler can interleave.
"""

import numpy as np
import ml_dtypes

import concourse.bacc as bacc
import concourse.tile as tile
from concourse import mybir

F32 = mybir.dt.float32
F32R = mybir.dt.float32r
BF16 = mybir.dt.bfloat16
AF = mybir.ActivationFunctionType
OP = mybir.AluOpType

# Model constants (hardcoded per contest contract)
V, E, NH, HS, FF, NB, L = 16, 512, 8, 64, 2048, 6, 512
B, T = 16, 512
TEMP = 1.0 * 0.8
EPS = 1e-5
SCALE = HS ** -0.5  # 0.125

NCORES = 8
SEQ = 2              # sequences per core
NTOK = SEQ * T       # 1024 tokens per core
C = E // 128         # 4 E-chunks
CF = FF // 128       # 16 FF-chunks
HP = NH // 2         # 4 head-pairs
NJ = T // 128        # 4 tk chunks per sequence
NJW = [T - 128 * j for j in range(NJ)]          # [512, 384, 256, 128]
POFF = [0, 512, 896, 1152]                      # compact score offsets

_PROGRAM_CACHE = {}


def _emit_ln_tt(nc, pools, h_t, ones_t, eps2_t, g_ap, b_ap, trivial, tt,
                hb, sq, xn, apply=True):
    """One token-tile of LayerNorm into caller-allocated hb/sq/xn tiles.
    With apply=False the normalize is not applied; returns (r_bf, z_bf) so
    the caller can fold the affine into a downstream matmul output."""
    stats, stats_bf = pools["stats"], pools["stats_bf"]
    ps1 = pools["ps1"]
    sl = slice(tt * 512, tt * 512 + 512)
    s1 = ps1.tile([128, 512], F32, tag="ps1", name="s1")
    s2 = ps1.tile([128, 512], F32, tag="ps1", name="s2")
    for c in range(C):
        nc.scalar.copy(hb[:, c, sl], h_t[:, c, sl])
        sq = pools["scr2"].tile([128, 512], BF16, tag="sq", name="sq")
        nc.vector.tensor_tensor(sq[:], hb[:, c, sl], hb[:, c, sl],
                                OP.mult)
        nc.tensor.matmul(s1[:], ones_t[:], hb[:, c, sl],
                         start=(c == 0), stop=(c == C - 1))
        nc.tensor.matmul(s2[:], ones_t[:], sq[:],
                         start=(c == 0), stop=(c == C - 1))
    msq = stats.tile([128, 512], F32, tag="stats")
    nc.scalar.square(msq[:], s1[:])
    var = stats.tile([128, 512], F32, tag="stats")
    nc.vector.scalar_tensor_tensor(out=var[:], in0=s2[:], scalar=float(E),
                                   in1=msq[:], op0=OP.mult, op1=OP.subtract)
    std = stats.tile([128, 512], F32, tag="stats")
    nc.scalar.activation(std[:], var[:], AF.Sqrt, bias=eps2_t[:])
    rc = stats.tile([128, 512], F32, tag="stats")
    nc.vector.reciprocal_approx_fast(out=rc[:], in_=std[:])
    r_bf = stats_bf.tile([128, 512], BF16, tag="r_bf")
    nc.scalar.mul(r_bf[:], rc[:], float(E))
    if not apply:
        return r_bf, s1
    z_bf = stats_bf.tile([128, 512], BF16, tag="z_bf")
    nc.vector.tensor_tensor(z_bf[:], s1[:], rc[:], OP.mult)
    for c in range(C):
        nc.vector.tensor_tensor(xn[:, c, sl], hb[:, c, sl], r_bf[:], OP.mult)
        nc.vector.tensor_tensor(xn[:, c, sl], xn[:, c, sl], z_bf[:],
                                OP.subtract)
        if not trivial:
            nc.vector.tensor_scalar(out=xn[:, c, sl], in0=xn[:, c, sl],
                                    scalar1=g_ap[:, c:c + 1],
                                    scalar2=b_ap[:, c:c + 1],
                                    op0=OP.mult, op1=OP.add)
    return None


def _alloc_ln(pools):
    hb = pools["scr"].tile([128, C, NTOK], BF16, tag="scratch", name="hb")
    xn = pools["scr"].tile([128, C, NTOK], BF16, tag="scratch", name="xnt")
    return hb, None, xn


def _emit_ln(nc, pools, h_t, ones_t, eps2_t, g_ap, b_ap, trivial):
    hb, sq, xn = _alloc_ln(pools)
    for tt in range(2):
        _emit_ln_tt(nc, pools, h_t, ones_t, eps2_t, g_ap, b_ap, trivial, tt,
                    hb, sq, xn)
    return xn


def build_program(ln_trivial, bias_trivial=None, nb_run=NB,
                  ln_general_params=True, ob_trivial=True):
    """Build the Bass program. ln_trivial: list of NB*2+1 bools (ln1/ln2 per
    layer then lnf) -- when True the g/b application op is skipped."""
    if bias_trivial is None:
        bias_trivial = [False] * NB
    nc = bacc.Bacc(None, target_bir_lowering=False)

    h0_d = nc.dram_tensor("h0", [128, C * NTOK], F32, kind="ExternalInput")
    wq_d = nc.dram_tensor("wq", [NB, 128, C * 512], BF16, kind="ExternalInput")
    wk_d = nc.dram_tensor("wk", [NB, 128, C * 512], BF16, kind="ExternalInput")
    wv_d = nc.dram_tensor("wv", [NB, 128, C * 512], BF16, kind="ExternalInput")
    pw_d = nc.dram_tensor("pw", [NB, 128, C * 512], BF16, kind="ExternalInput")
    f1_d = nc.dram_tensor("f1", [NB, 128, C * FF], BF16, kind="ExternalInput")
    f2_d = nc.dram_tensor("f2", [NB, 128, CF * 512], BF16, kind="ExternalInput")
    pb_d = nc.dram_tensor("pb", [128, NB * C], F32, kind="ExternalInput")
    fb1_d = nc.dram_tensor("fb1", [128, NB * CF], F32, kind="ExternalInput")
    fb2_d = nc.dram_tensor("fb2", [128, NB * C], F32, kind="ExternalInput")
    ow_d = nc.dram_tensor("ow", [128, C * V], BF16, kind="ExternalInput")
    ob_d = nc.dram_tensor("ob", [V, 1], F32, kind="ExternalInput")
    ncs_d = nc.dram_tensor("ncs", [V, 1], F32, kind="ExternalInput")
    tri_d = nc.dram_tensor("tri", [128, 128], BF16, kind="ExternalInput")
    lng_d = lnb_d = None
    if ln_general_params:
        lng_d = nc.dram_tensor("lng", [128, (2 * NB + 1) * C], F32,
                               kind="ExternalInput")
        lnb_d = nc.dram_tensor("lnb", [128, (2 * NB + 1) * C], F32,
                               kind="ExternalInput")
    out_d = nc.dram_tensor("logits", [V, NTOK], F32, kind="ExternalOutput")

    from contextlib import ExitStack
    with ExitStack() as ctx:
        tc = ctx.enter_context(tile.TileContext(nc))
        consts = ctx.enter_context(tc.tile_pool(name="consts", bufs=1))
        hpool = ctx.enter_context(tc.tile_pool(name="hpool", bufs=1))
        wqkv = ctx.enter_context(tc.tile_pool(name="wqkv", bufs=1))
        wff1 = ctx.enter_context(tc.tile_pool(name="wff1", bufs=2))
        wff2 = ctx.enter_context(tc.tile_pool(name="wff2", bufs=1))
        scr = ctx.enter_context(tc.tile_pool(name="scr", bufs=4))
        scr2 = ctx.enter_context(tc.tile_pool(name="scr2", bufs=2))
        qk = ctx.enter_context(tc.tile_pool(name="qk", bufs=2))
        vt = ctx.enter_context(tc.tile_pool(name="vt", bufs=1))
        pp = ctx.enter_context(tc.tile_pool(name="pp", bufs=2))
        osb = ctx.enter_context(tc.tile_pool(name="osb", bufs=1))
        ffa = ctx.enter_context(tc.tile_pool(name="ffa", bufs=2))
        stats = ctx.enter_context(tc.tile_pool(name="stats", bufs=6))
        stats_bf = ctx.enter_context(tc.tile_pool(name="stats_bf", bufs=2))
        ps1 = ctx.enter_context(tc.tile_pool(name="ps1", bufs=4, space="PSUM"))
        psA = ctx.enter_context(tc.tile_pool(name="psA", bufs=2, space="PSUM"))
        ps2 = ps1

        pools = {"scr": scr, "scr2": scr2, "stats": stats,
                 "stats_bf": stats_bf, "ps2": ps2, "ps1": ps1}

        h_t = hpool.tile([128, C, NTOK], F32)
        for c in range(C):
            nc.sync.dma_start(h_t[:, c, 0:512], h0_d[:].rearrange(
                "p (c t) -> p c t", t=NTOK)[:, c, 0:512])

        # layer-0 weights hoisted so wv/wq/wk transfer before h0's second
        # token-tile and long before the consts
        def load_w(nm, d_t, pool, kc, m, i=0):
            t = pool.tile([128, kc, m], BF16, tag=nm)
            nc.sync.dma_start(t[:], d_t[i].rearrange(
                "p (c m) -> p c m", m=m))
            return t

        w_l0 = {"wv": load_w("wv", wv_d, wqkv, C, 512),
                "wq": load_w("wq", wq_d, wqkv, C, 512),
                "wk": load_w("wk", wk_d, wqkv, C, 512)}
        for c in range(C):
            nc.sync.dma_start(h_t[:, c, 512:1024], h0_d[:].rearrange(
                "p (c t) -> p c t", t=NTOK)[:, c, 512:1024])
        w_l0["pw"] = load_w("pw", pw_d, wqkv, C, 512)
        w_l0["f1"] = load_w("f1", f1_d, wff1, C, FF)
        w_l0["f2"] = load_w("f2", f2_d, wff2, CF, 512)

        ones_t = consts.tile([128, 128], BF16)
        nc.gpsimd.memset(ones_t[:], 1.0)
        eps2_t = consts.tile([128, 1], F32)
        nc.gpsimd.memset(eps2_t[:], float(E) * float(E) * EPS)
        tri_t = consts.tile([128, 128], BF16)
        nc.sync.dma_start(tri_t[:], tri_d[:])
        pb_t = consts.tile([128, NB * C], F32)
        nc.sync.dma_start(pb_t[:], pb_d[:])
        fb1_t = consts.tile([128, NB * CF], F32)
        nc.sync.dma_start(fb1_t[:], fb1_d[:])
        fb2_t = consts.tile([128, NB * C], F32)
        nc.sync.dma_start(fb2_t[:], fb2_d[:])
        ow_t = consts.tile([128, C, V], BF16)
        nc.sync.dma_start(ow_t[:], ow_d[:].rearrange("p (c v) -> p c v", v=V))
        ob_t = consts.tile([V, 1], F32)
        nc.sync.dma_start(ob_t[:], ob_d[:])
        ncs_t = consts.tile([V, 1], F32)
        nc.sync.dma_start(ncs_t[:], ncs_d[:])
        lng_t = lnb_t = None
        if ln_general_params:
            lng_t = consts.tile([128, 2 * NB + 1, C], F32)
            nc.sync.dma_start(lng_t[:], lng_d[:].rearrange(
                "p (l c) -> p l c", c=C))
            lnb_t = consts.tile([128, 2 * NB + 1, C], F32)
            nc.sync.dma_start(lnb_t[:], lnb_d[:].rearrange(
                "p (l c) -> p l c", c=C))

        # V tile: per key-block slot: [ones|v_h0|v_h1] x HP + trailing ones.
        # AV matmuls use a 128-wide lhsT = [ones|v_h0] (h2=0) or
        # [v_h1|ones-of-next-block] (h2=1) so the otherwise-idle half of the
        # PE array emits the softmax denominator (broadcast 64x) in the same
        # pass -- no denominator matmuls at all. h0's o lands on rows 64:128
        # and h1's on rows 0:64; the proj weights are permuted host-side to
        # match. v_h0/v_h1 are adjacent so the V copy is one strided copy
        # with 128-wide runs per half.
        vt_t = vt.tile([128, SEQ * NJ, HP * 192 + 64], BF16, tag="vt")
        for hp0 in range(HP + 1):
            nc.gpsimd.memset(
                vt_t[:, :, hp0 * 192:hp0 * 192 + 64], 1.0)

        def ln_params(idx):
            if ln_general_params and not ln_trivial[idx]:
                return lng_t[:, idx, :], lnb_t[:, idx, :], False
            return None, None, True

        lnf_stats = [None, None]
        for i in range(nb_run):
            # ---- this layer's weights (wv first: V is consumed first) ----
            if i == 0:
                wv_t, wq_t, wk_t = w_l0["wv"], w_l0["wq"], w_l0["wk"]
                pw_t, f1_t, f2_t = w_l0["pw"], w_l0["f1"], w_l0["f2"]
            else:
                wv_t = load_w("wv", wv_d, wqkv, C, 512, i)
                wq_t = load_w("wq", wq_d, wqkv, C, 512, i)
                wk_t = load_w("wk", wk_d, wqkv, C, 512, i)
                pw_t = load_w("pw", pw_d, wqkv, C, 512, i)
                f1_t = load_w("f1", f1_d, wff1, C, FF, i)
                f2_t = load_w("f2", f2_d, wff2, CF, 512, i)

            # ---- V projection, token-major: vT[tk, hd*64+d] ----
            def emit_vpair(jgp):
                vp = psA.tile([128, 2, 512], F32, tag="psA")
                for half in range(2):
                    jg = 2 * jgp + half
                    for c in range(C):
                        nc.tensor.matmul(vp[:, half],
                                         xn[:, c, jg * 128:(jg + 1) * 128],
                                         wv_t[:, c, :],
                                         start=(c == 0), stop=(c == C - 1))
                for half in range(2):
                    jg = 2 * jgp + half
                    src = vp[:, half, :].rearrange("p (h x) -> p h x", x=128)
                    dst = vt_t[:, jg, 0:HP * 192].rearrange(
                        "p (h x) -> p h x", x=192)
                    nc.scalar.copy(dst[:, :, 64:192], src[:])

            # ---- LN1 (layer 0: emitted here, with the tt0 V-pairs between
            #      the two token-tiles so stats(tt1) can wait on its h0 DMA
            #      without idling the PE; others peeled into the previous
            #      layer's FFN emission) ----
            # tt1 V-pairs deferred into the hp loop so the layer-boundary
            # peel-LN chain gets matmul cover that does not depend on
            # apply(tt1).
            if i == 0:
                g_ap, b_ap, triv = ln_params(0)
                ln1 = _alloc_ln(pools)
                _emit_ln_tt(nc, pools, h_t, ones_t, eps2_t, g_ap, b_ap,
                            triv, 0, *ln1)
                xn = ln1[2]
                emit_vpair(0)
                emit_vpair(1)
                _emit_ln_tt(nc, pools, h_t, ones_t, eps2_t, g_ap, b_ap,
                            triv, 1, *ln1)
            else:
                xn = xn_next
                emit_vpair(0)
                emit_vpair(1)

            o_t = osb.tile([128, C, NTOK], BF16, tag="o")

            def emit_den_o(s, hp, p_t):
                base = s * T
                # h2=0: lhsT [ones|v_h0] -> rows 0:64 = den, 64:128 = o
                # h2=1: lhsT [v_h1|ones'] -> rows 0:64 = o, 64:128 = den
                ops = []
                for h2 in range(2):
                    vb = hp * 192 + 128 * h2
                    op = ps1.tile([128, 512], F32, tag="ps1")
                    for j in range(NJ):
                        off = j * 128
                        njw = T - off
                        nc.tensor.matmul(
                            op[:, off:T],
                            vt_t[:, s * NJ + j, vb:vb + 128],
                            p_t[:, h2, POFF[j]:POFF[j] + njw],
                            start=(j == 0), stop=(j == NJ - 1))
                    ops.append(op)
                opA, opB = ops
                # reciprocals must run at base partition 0; cross-half copies
                # (DVE bank0 -> any half) align each den with its numerator
                rdA0 = stats.tile([128, 512], F32, tag="stats")
                nc.vector.reciprocal_approx_fast(out=rdA0[0:64, :],
                                                 in_=opA[0:64, :])
                rdA = stats.tile([128, 512], F32, tag="stats")
                nc.vector.tensor_copy(rdA[64:128, :], rdA0[0:64, :])
                dB = stats.tile([128, 512], F32, tag="stats")
                nc.vector.tensor_copy(dB[0:64, :], opB[64:128, :])
                rdB = stats.tile([128, 512], F32, tag="stats")
                nc.vector.reciprocal_approx_fast(out=rdB[0:64, :],
                                                 in_=dB[0:64, :])
                nc.vector.tensor_tensor(
                    o_t[64:128, hp, base:base + T], opA[64:128, 0:T],
                    rdA[64:128, :], OP.mult)
                nc.vector.tensor_tensor(
                    o_t[0:64, hp, base:base + T], opB[0:64, 0:T],
                    rdB[0:64, :], OP.mult)

            pending = None
            for hp in range(HP):
                msl = slice(hp * 128, (hp + 1) * 128)
                q_t = qk.tile([128, NTOK], BF16, tag="q")
                k_t = qk.tile([128, NTOK], BF16, tag="k")
                for tt in range(2):
                    sl = slice(tt * 512, tt * 512 + 512)
                    qp = ps1.tile([128, 512], F32, tag="ps1")
                    kp = ps1.tile([128, 512], F32, tag="ps1")
                    for c in range(C):
                        nc.tensor.matmul(qp[:], wq_t[:, c, msl],
                                         xn[:, c, sl],
                                         start=(c == 0), stop=(c == C - 1))
                        nc.tensor.matmul(kp[:], wk_t[:, c, msl],
                                         xn[:, c, sl],
                                         start=(c == 0), stop=(c == C - 1))
                    if hp == 0:
                        # layer start: the vector queue still drains the
                        # previous layer's peel-LN chain; scalar is lighter
                        nc.scalar.copy(q_t[:, sl], qp[:])
                    else:
                        nc.vector.tensor_copy(q_t[:, sl], qp[:])
                    nc.scalar.copy(k_t[:, sl], kp[:])

                for s in range(SEQ):
                    base = s * T
                    p_t = pp.tile([128, 2, 1408], BF16, tag="p")
                    for h2 in range(2):
                        dsl = slice(h2 * 64, h2 * 64 + 64)
                        sA = psA.tile([128, 1024], F32, tag="psA")
                        sB = ps1.tile([128, 512], F32, tag="ps1")
                        locs = [sA[:, 0:512], sA[:, 512:896],
                                sB[:, 0:256], sB[:, 256:384]]
                        for j in range(NJ):
                            off = j * 128
                            nc.tensor.matmul(
                                locs[j],
                                k_t[dsl, base + off:base + off + 128],
                                q_t[dsl, base + off:base + T],
                                start=True, stop=True)
                        nc.scalar.activation(
                            p_t[:, h2, 0:896], sA[:, 0:896], AF.Exp,
                            scale=SCALE)
                        nc.scalar.activation(
                            p_t[:, h2, 896:1280], sB[:, 0:384], AF.Exp,
                            scale=SCALE)
                    m01 = p_t[:, :, 0:1024].rearrange(
                        "p h (j c) -> p h j c", c=512)[:, :, :, 0:128]
                    m23 = p_t[:, :, 896:1408].rearrange(
                        "p h (j c) -> p h j c", c=256)[:, :, :, 0:128]
                    trib = tri_t[:, None, None, :].to_broadcast(
                        (128, 2, 2, 128))
                    nc.vector.tensor_tensor(m01, m01, trib, OP.mult)
                    nc.vector.tensor_tensor(m23, m23, trib, OP.mult)
                    if pending is not None:
                        emit_den_o(*pending)
                    pending = (s, hp, p_t)
                    if hp == 0 and s == 0:
                        emit_vpair(2)
                        emit_vpair(3)

            # ---- attention out projection + residual, interleaved with
            #      LN2 so proj(tt1)'s matmuls cover LN2(tt0)'s chain ----
            def emit_proj(tt):
                sl = slice(tt * 512, tt * 512 + 512)
                for mc in range(C):
                    pj = ps1.tile([128, 512], F32, tag="ps1")
                    for c in range(C):
                        nc.tensor.matmul(pj[:],
                                         pw_t[:, c, mc * 128:(mc + 1) * 128],
                                         o_t[:, c, sl],
                                         start=(c == 0), stop=(c == C - 1))
                    nc.vector.scalar_tensor_tensor(
                        out=h_t[:, mc, sl], in0=pj[:],
                        scalar=pb_t[:, i * C + mc:i * C + mc + 1],
                        in1=h_t[:, mc, sl], op0=OP.add, op1=OP.add)

            g_ap, b_ap, triv = ln_params(2 * i + 1)
            ln2 = _alloc_ln(pools)
            emit_proj(0)
            # flush the last AV after proj(0): its matmuls cover the LN2(tt0)
            # scalar/vector chain (proj(0) only needs o_t token-tile 0)
            emit_den_o(*pending)
            _emit_ln_tt(nc, pools, h_t, ones_t, eps2_t, g_ap, b_ap,
                        triv, 0, *ln2)
            emit_proj(1)
            _emit_ln_tt(nc, pools, h_t, ones_t, eps2_t, g_ap, b_ap,
                        triv, 1, *ln2)
            xn2 = ln2[2]

            for tt in range(2):
                sl = slice(tt * 512, tt * 512 + 512)
                fa = ffa.tile([128, CF, 512], BF16, tag="fa")
                for mfp in range(CF // 2):
                    fp = psA.tile([128, 2, 512], F32, tag="psA")
                    for half in range(2):
                        mf = 2 * mfp + half
                        for c in range(C):
                            nc.tensor.matmul(
                                fp[:, half],
                                f1_t[:, c, mf * 128:(mf + 1) * 128],
                                xn2[:, c, sl],
                                start=(c == 0), stop=(c == C - 1))
                    if bias_trivial[i]:
                        nc.scalar.activation(
                            fa[:, 2 * mfp:2 * mfp + 2, :], fp[:], AF.Relu)
                    else:
                        for half in range(2):
                            mf = 2 * mfp + half
                            nc.scalar.activation(
                                fa[:, mf, :], fp[:, half], AF.Relu,
                                bias=fb1_t[:, i * CF + mf:i * CF + mf + 1])
                for mc in range(C):
                    f2p = ps1.tile([128, 512], F32, tag="ps1")
                    for c16 in range(CF):
                        nc.tensor.matmul(f2p[:],
                                         f2_t[:, c16, mc * 128:(mc + 1) * 128],
                                         fa[:, c16, :],
                                         start=(c16 == 0),
                                         stop=(c16 == CF - 1))
                    nc.vector.scalar_tensor_tensor(
                        out=h_t[:, mc, sl], in0=f2p[:],
                        scalar=fb2_t[:, i * C + mc:i * C + mc + 1],
                        in1=h_t[:, mc, sl], op0=OP.add, op1=OP.add)
                # peel next layer's LN1(tt) (or the final LN on the last
                # layer) here so its scalar/vector chain hides behind the
                # other token-tile's FFN matmuls
                if i + 1 < nb_run:
                    if tt == 0:
                        ln_next = _alloc_ln(pools)
                    g_ap, b_ap, triv = ln_params(2 * (i + 1))
                    _emit_ln_tt(nc, pools, h_t, ones_t, eps2_t, g_ap, b_ap,
                                triv, tt, *ln_next)
                    if tt == 1:
                        xn_next = ln_next[2]
                elif nb_run == NB:
                    # final LN: stats only; the normalize affine is folded
                    # into the logits output: logits = r*(P - s1*cs/E) (+ob)
                    # where P = hb@ow. t1 = P - s1*cs/E is computed as soon
                    # as the stats land so only r (std->recip chain) remains
                    # on the tail.
                    def emit_logits_pre(ltt):
                        sl2 = slice(ltt * 512, ltt * 512 + 512)
                        s1c = lnf_stats[ltt][1]
                        lg = ps1.tile([V, 512], F32, tag="ps1")
                        for c in range(C):
                            nc.tensor.matmul(lg[:], ow_t[:, c, :],
                                             ln_next[0][:, c, sl2],
                                             start=(c == 0),
                                             stop=(c == C - 1))
                        t1 = stats_bf.tile([128, 512], BF16, tag="lgt")
                        nc.vector.scalar_tensor_tensor(
                            out=t1[0:V, :], in0=s1c[0:V, :], scalar=ncs_t[:],
                            in1=lg[:], op0=OP.mult, op1=OP.add)
                        lnf_stats[ltt] = (lnf_stats[ltt][0], t1)
                    if tt == 0:
                        ln_next = _alloc_ln(pools)
                    else:
                        emit_logits_pre(0)
                    r_bf, s1 = _emit_ln_tt(
                        nc, pools, h_t, ones_t, eps2_t, None, None, True,
                        tt, *ln_next, apply=False)
                    s1c = stats.tile([128, 512], F32, tag="stats")
                    nc.scalar.copy(s1c[0:V, :], s1[0:V, :])
                    lnf_stats[tt] = (r_bf, s1c)
                    if tt == 1:
                        emit_logits_pre(1)

        # ---- logits tail: out = t1 * r (+ ob) ----
        if nb_run == NB:
            for tt in range(2):
                sl = slice(tt * 512, tt * 512 + 512)
                r_bf, t1 = lnf_stats[tt]
                lgs = stats.tile([128, 512], F32, tag="stats")
                nc.vector.tensor_tensor(lgs[0:V, :], t1[0:V, :],
                                        r_bf[0:V, :], OP.mult)
                if not ob_trivial:
                    nc.vector.tensor_scalar_add(lgs[0:V, :], lgs[0:V, :],
                                                ob_t[:])
                nc.sync.dma_start(out_d[:, sl], lgs[0:V, :])
        else:
            xnf = _emit_ln(nc, pools, h_t, ones_t, eps2_t, None, None, True)
            for tt in range(2):
                sl = slice(tt * 512, tt * 512 + 512)
                lg = ps1.tile([V, 512], F32, tag="ps1")
                lgs = stats.tile([128, 512], F32, tag="stats")
                for c in range(C):
                    nc.tensor.matmul(lg[:], ow_t[:, c, :], xnf[:, c, sl],
                                     start=(c == 0), stop=(c == C - 1))
                nc.vector.tensor_scalar_add(lgs[0:V, :], lg[:], ob_t[:])
                nc.sync.dma_start(out_d[:, sl], lgs[0:V, :])

    nc.finalize()
    return nc


def prepare_inputs(inputs):
    """Host-side preprocessing: embedding gather, weight layout + bf16 cast.
    Returns (shared_map, per_core_h0_list, ln_trivial, bias_trivial)."""
    f32 = np.float32
    bf16 = ml_dtypes.bfloat16
    x = np.asarray(inputs["x"]).astype(np.int64)
    emb = np.asarray(inputs["emb"], dtype=f32)
    pos = np.asarray(inputs["pos"], dtype=f32)

    positions = np.minimum(np.arange(T), L - 1)
    h0 = emb[x] + pos[positions][None, :, :]      # [B, T, E] fp32

    def to_dev_lhst(mat, kchunks, mcols):
        m = np.ascontiguousarray(mat.astype(bf16))
        return m.reshape(kchunks, 128, mcols).transpose(1, 0, 2).reshape(
            128, kchunks * mcols)

    wq = np.asarray(inputs["wq"], dtype=f32)
    wk = np.asarray(inputs["wk"], dtype=f32)
    wv = np.asarray(inputs["wv"], dtype=f32)
    pw = np.asarray(inputs["proj_w"], dtype=f32)
    f1 = np.asarray(inputs["ff_w1"], dtype=f32)
    f2 = np.asarray(inputs["ff_w2"], dtype=f32)

    wq_dev = np.stack([to_dev_lhst(wq[i].transpose(1, 0, 2).reshape(E, NH * HS),
                                   C, 512) for i in range(NB)])
    wk_dev = np.stack([to_dev_lhst(wk[i].transpose(1, 0, 2).reshape(E, NH * HS),
                                   C, 512) for i in range(NB)])
    wv_dev = np.stack([to_dev_lhst(wv[i].transpose(1, 0, 2).reshape(E, NH * HS),
                                   C, 512) for i in range(NB)])
    # o_t holds [h1|h0] per head-pair chunk (see vt layout in build_program):
    # permute proj_w rows to match
    pw_perm = np.concatenate([
        np.concatenate([np.arange(hp * 128 + 64, hp * 128 + 128),
                        np.arange(hp * 128, hp * 128 + 64)])
        for hp in range(HP)])
    pw_dev = np.stack([to_dev_lhst(pw[i][pw_perm], C, 512)
                       for i in range(NB)])
    f1_dev = np.stack([to_dev_lhst(f1[i], C, FF) for i in range(NB)])
    f2_dev = np.stack([to_dev_lhst(f2[i], CF, 512) for i in range(NB)])

    def vec_dev(v, chunks):
        return np.ascontiguousarray(v.astype(f32).reshape(chunks, 128).T)

    pb_dev = np.concatenate([vec_dev(np.asarray(inputs["proj_b"][i]), C)
                             for i in range(NB)], axis=1)
    fb1_dev = np.concatenate([vec_dev(np.asarray(inputs["ff_b1"][i]), CF)
                              for i in range(NB)], axis=1)
    fb2_dev = np.concatenate([vec_dev(np.asarray(inputs["ff_b2"][i]), C)
                              for i in range(NB)], axis=1)
    # fold the final-LN affine into the logits head:
    # logits = LNstat(h)*g@W/T + b@W/T + ob/T = r*(h@W') - mu*colsum(W') + ob'
    lnf_g_v = np.asarray(inputs["lnf_g"], dtype=f32)
    lnf_b_v = np.asarray(inputs["lnf_b"], dtype=f32)
    out_w_v = np.asarray(inputs["out_w"], dtype=f32)
    ow_eff = (lnf_g_v[:, None] * out_w_v) / TEMP
    ow_dev = to_dev_lhst(ow_eff, C, V)
    ncs_dev = np.ascontiguousarray((-ow_eff.sum(axis=0) / E).reshape(V, 1))
    ob_eff = (np.asarray(inputs["out_b"], dtype=f32)
              + lnf_b_v @ out_w_v) / TEMP
    ob_dev = np.ascontiguousarray(ob_eff.reshape(V, 1))
    ob_trivial = bool(np.all(ob_eff == 0.0))
    tri_dev = np.triu(np.ones((128, 128), dtype=f32)).astype(bf16)

    gs, bs, ln_trivial = [], [], []
    for i in range(NB):
        for nm_g, nm_b in (("ln1_g", "ln1_b"), ("ln2_g", "ln2_b")):
            g = np.asarray(inputs[nm_g][i], dtype=f32)
            b = np.asarray(inputs[nm_b][i], dtype=f32)
            gs.append(vec_dev(g, C))
            bs.append(vec_dev(b, C))
            ln_trivial.append(bool(np.all(g == 1.0) and np.all(b == 0.0)))
    g = np.asarray(inputs["lnf_g"], dtype=f32)
    b = np.asarray(inputs["lnf_b"], dtype=f32)
    gs.append(vec_dev(g, C))
    bs.append(vec_dev(b, C))
    ln_trivial.append(bool(np.all(g == 1.0) and np.all(b == 0.0)))
    lng_dev = np.concatenate(gs, axis=1)
    lnb_dev = np.concatenate(bs, axis=1)

    ln_trivial.append(ob_trivial)   # threaded through to build_program

    shared = {
        "wq": wq_dev, "wk": wk_dev, "wv": wv_dev, "pw": pw_dev,
        "f1": f1_dev, "f2": f2_dev, "pb": pb_dev, "fb1": fb1_dev,
        "fb2": fb2_dev, "ow": ow_dev, "ob": ob_dev, "ncs": ncs_dev,
        "tri": tri_dev, "lng": lng_dev, "lnb": lnb_dev,
    }

    h0_cores = []
    for core in range(NCORES):
        hh = h0[SEQ * core:SEQ * core + SEQ]          # [SEQ, T, E]
        hT = hh.transpose(2, 0, 1).reshape(E, NTOK)   # [E, NTOK]
        h0_cores.append(np.ascontiguousarray(
            hT.reshape(C, 128, NTOK).transpose(1, 0, 2).reshape(
                128, C * NTOK)))
    bias_trivial = []
    for i in range(NB):
        bias_trivial.append(bool(
            np.all(np.asarray(inputs["proj_b"][i]) == 0.0)
            and np.all(np.asarray(inputs["ff_b1"][i]) == 0.0)
            and np.all(np.asarray(inputs["ff_b2"][i]) == 0.0)))
    return shared, h0_cores, ln_trivial, bias_trivial


def assemble_output(core_logits):
    """core_logits: list of [V, NTOK] fp32 -> [B, T, V]."""
    out = np.empty((B, T, V), np.float32)
    for core in range(NCORES):
        lg = core_logits[core].reshape(V, SEQ, T)
        out[SEQ * core:SEQ * core + SEQ] = lg.transpose(1, 2, 0)
    return out


def get_program(ln_trivial, bias_trivial):
    key = (tuple(ln_trivial), tuple(bias_trivial))
    if key not in _PROGRAM_CACHE:
        lt = list(key[0])
        _PROGRAM_CACHE[key] = build_program(
            lt[:2 * NB + 1], list(key[1]), ob_trivial=bool(lt[-1]))
    return _PROGRAM_CACHE[key]


def reset_device():
    """Recover a wedged accelerator (axon session reset). Best-effort."""
    try:
        import ctypes
        import jax
        jax.devices()
        lib = ctypes.CDLL('/opt/axon/libaxon_pjrt.so')
        lib.axon_reset.restype = ctypes.c_int64
        lib.axon_reset()
    except Exception:
        pass


def kernel(**inputs):
    from concourse.bass_utils import run_bass_kernel_spmd
    shared, h0_cores, ln_trivial, bias_trivial = prepare_inputs(inputs)
    nc = get_program(ln_trivial, bias_trivial)
    in_maps = [dict(shared, h0=h0_cores[c]) for c in range(NCORES)]
    try:
        res = run_bass_kernel_spmd(nc, in_maps, core_ids=list(range(NCORES)))
    except Exception:
        # A previous (profiled) session can leave the device wedged; reset
        # the axon session and retry once.
        reset_device()
        res = run_bass_kernel_spmd(nc, in_maps, core_ids=list(range(NCORES)))
    return assemble_output([res.results[c]["logits"] for c in range(NCORES)])



# revision 83
# speedup vs baseline: 1.0048x; 1.0048x over previous
"""Trainium2 Bass kernel for nn_AutoregressiveArithmeticTransformer.

6-layer dense transformer: B=16, T=512, E=512, NH=8 heads x HS=64, FF=2048,
V=16, causal attention, pre-LN, learned abacus embedding, logits / 0.8.

Strategy: data-parallel over batch across 8 NeuronCores (2 sequences per
core, no collectives). Activations live feature-major in SBUF
([E-partitions, tokens]); weights are streamed per-layer in bf16; all
matmuls run in bf16 with fp32 PSUM accumulation; the residual stream stays
fp32. LayerNorm statistics are computed with ones-matmuls on the PE;
attention scores are computed transposed ([tk, tq]) so the softmax
denominator is also a ones-matmul; V is produced token-major directly so
no transposes are ever needed.

All ops are token-tile (512) granular so the two sequences per core form
independent dependency streams the Tile scheduler can interleave.
"""

import numpy as np
import ml_dtypes

import concourse.bacc as bacc
import concourse.tile as tile
from concourse import mybir

F32 = mybir.dt.float32
F32R = mybir.dt.float32r
BF16 = mybir.dt.bfloat16
AF = mybir.ActivationFunctionType
OP = mybir.AluOpType

# Model constants (hardcoded per contest contract)
V, E, NH, HS, FF, NB, L = 16, 512, 8, 64, 2048, 6, 512
B, T = 16, 512
TEMP = 1.0 * 0.8
EPS = 1e-5
SCALE = HS ** -0.5  # 0.125

NCORES = 8
SEQ = 2              # sequences per core
NTOK = SEQ * T       # 1024 tokens per core
C = E // 128         # 4 E-chunks
CF = FF // 128       # 16 FF-chunks
HP = NH // 2         # 4 head-pairs
NJ = T // 128        # 4 tk chunks per sequence
NJW = [T - 128 * j for j in range(NJ)]          # [512, 384, 256, 128]
POFF = [0, 512, 896, 1152]                      # compact score offsets

_PROGRAM_CACHE = {}


def _emit_ln_tt(nc, pools, h_t, ones_t, eps2_t, g_ap, b_ap, trivial, tt,
                hb, sq, xn, apply=True):
    """One token-tile of LayerNorm into caller-allocated hb/sq/xn tiles.
    With apply=False the normalize is not applied; returns (r_bf, z_bf) so
    the caller can fold the affine into a downstream matmul output."""
    stats, stats_bf = pools["stats"], pools["stats_bf"]
    ps1 = pools["ps1"]
    sl = slice(tt * 512, tt * 512 + 512)
    s1 = ps1.tile([128, 512], F32, tag="ps1", name="s1")
    s2 = ps1.tile([128, 512], F32, tag="ps1", name="s2")
    for c in range(C):
        nc.scalar.copy(hb[:, c, sl], h_t[:, c, sl])
        sq = pools["scr2"].tile([128, 512], BF16, tag="sq", name="sq")
        nc.vector.tensor_tensor(sq[:], hb[:, c, sl], hb[:, c, sl],
                                OP.mult)
        nc.tensor.matmul(s1[:], ones_t[:], hb[:, c, sl],
                         start=(c == 0), stop=(c == C - 1))
        nc.tensor.matmul(s2[:], ones_t[:], sq[:],
                         start=(c == 0), stop=(c == C - 1))
    msq = stats.tile([128, 512], F32, tag="stats")
    nc.scalar.square(msq[:], s1[:])
    var = stats.tile([128, 512], F32, tag="stats")
    nc.vector.scalar_tensor_tensor(out=var[:], in0=s2[:], scalar=float(E),
                                   in1=msq[:], op0=OP.mult, op1=OP.subtract)
    std = stats.tile([128, 512], F32, tag="stats")
    nc.scalar.activation(std[:], var[:], AF.Sqrt, bias=eps2_t[:])
    rc = stats.tile([128, 512], F32, tag="stats")
    nc.vector.reciprocal_approx_fast(out=rc[:], in_=std[:])
    r_bf = stats_bf.tile([128, 512], BF16, tag="r_bf")
    nc.scalar.mul(r_bf[:], rc[:], float(E))
    if not apply:
        return r_bf, s1
    z_bf = stats_bf.tile([128, 512], BF16, tag="z_bf")
    nc.vector.tensor_tensor(z_bf[:], s1[:], rc[:], OP.mult)
    for c in range(C):
        nc.vector.tensor_tensor(xn[:, c, sl], hb[:, c, sl], r_bf[:], OP.mult)
        nc.vector.tensor_tensor(xn[:, c, sl], xn[:, c, sl], z_bf[:],
                                OP.subtract)
        if not trivial:
            nc.vector.tensor_scalar(out=xn[:, c, sl], in0=xn[:, c, sl],
                                    scalar1=g_ap[:, c:c + 1],
                                    scalar2=b_ap[:, c:c + 1],
                                    op0=OP.mult, op1=OP.add)
    return None


def _alloc_ln(pools):
    hb = pools["scr"].tile([128, C, NTOK], BF16, tag="scratch", name="hb")
    xn = pools["scr"].tile([128, C, NTOK], BF16, tag="scratch", name="xnt")
    return hb, None, xn


def _emit_ln(nc, pools, h_t, ones_t, eps2_t, g_ap, b_ap, trivial):
    hb, sq, xn = _alloc_ln(pools)
    for tt in range(2):
        _emit_ln_tt(nc, pools, h_t, ones_t, eps2_t, g_ap, b_ap, trivial, tt,
                    hb, sq, xn)
    return xn


def build_program(ln_trivial, bias_trivial=None, nb_run=NB,
                  ln_general_params=True, ob_trivial=True):
    """Build the Bass program. ln_trivial: list of NB*2+1 bools (ln1/ln2 per
    layer then lnf) -- when True the g/b application op is skipped."""
    if bias_trivial is None:
        bias_trivial = [False] * NB
    nc = bacc.Bacc(None, target_bir_lowering=False)

    h0_d = nc.dram_tensor("h0", [128, C * NTOK], F32, kind="ExternalInput")
    wq_d = nc.dram_tensor("wq", [NB, 128, C * 512], BF16, kind="ExternalInput")
    wk_d = nc.dram_tensor("wk", [NB, 128, C * 512], BF16, kind="ExternalInput")
    wv_d = nc.dram_tensor("wv", [NB, 128, C * 512], BF16, kind="ExternalInput")
    pw_d = nc.dram_tensor("pw", [NB, 128, C * 512], BF16, kind="ExternalInput")
    f1_d = nc.dram_tensor("f1", [NB, 128, C * FF], BF16, kind="ExternalInput")
    f2_d = nc.dram_tensor("f2", [NB, 128, CF * 512], BF16, kind="ExternalInput")
    pb_d = nc.dram_tensor("pb", [128, NB * C], F32, kind="ExternalInput")
    fb1_d = nc.dram_tensor("fb1", [128, NB * CF], F32, kind="ExternalInput")
    fb2_d = nc.dram_tensor("fb2", [128, NB * C], F32, kind="ExternalInput")
    ow_d = nc.dram_tensor("ow", [128, C * V], BF16, kind="ExternalInput")
    ob_d = nc.dram_tensor("ob", [V, 1], F32, kind="ExternalInput")
    ncs_d = nc.dram_tensor("ncs", [V, 1], F32, kind="ExternalInput")
    tri_d = nc.dram_tensor("tri", [128, 128], BF16, kind="ExternalInput")
    lng_d = lnb_d = None
    if ln_general_params:
        lng_d = nc.dram_tensor("lng", [128, (2 * NB + 1) * C], F32,
                               kind="ExternalInput")
        lnb_d = nc.dram_tensor("lnb", [128, (2 * NB + 1) * C], F32,
                               kind="ExternalInput")
    out_d = nc.dram_tensor("logits", [V, NTOK], F32, kind="ExternalOutput")

    from contextlib import ExitStack
    with ExitStack() as ctx:
        tc = ctx.enter_context(tile.TileContext(nc))
        consts = ctx.enter_context(tc.tile_pool(name="consts", bufs=1))
        hpool = ctx.enter_context(tc.tile_pool(name="hpool", bufs=1))
        wqkv = ctx.enter_context(tc.tile_pool(name="wqkv", bufs=1))
        wff1 = ctx.enter_context(tc.tile_pool(name="wff1", bufs=2))
        wff2 = ctx.enter_context(tc.tile_pool(name="wff2", bufs=1))
        scr = ctx.enter_context(tc.tile_pool(name="scr", bufs=4))
        scr2 = ctx.enter_context(tc.tile_pool(name="scr2", bufs=2))
        qk = ctx.enter_context(tc.tile_pool(name="qk", bufs=2))
        vt = ctx.enter_context(tc.tile_pool(name="vt", bufs=1))
        pp = ctx.enter_context(tc.tile_pool(name="pp", bufs=2))
        osb = ctx.enter_context(tc.tile_pool(name="osb", bufs=1))
        ffa = ctx.enter_context(tc.tile_pool(name="ffa", bufs=2))
        stats = ctx.enter_context(tc.tile_pool(name="stats", bufs=6))
        stats_bf = ctx.enter_context(tc.tile_pool(name="stats_bf", bufs=2))
        ps1 = ctx.enter_context(tc.tile_pool(name="ps1", bufs=4, space="PSUM"))
        psA = ctx.enter_context(tc.tile_pool(name="psA", bufs=2, space="PSUM"))
        ps2 = ps1

        pools = {"scr": scr, "scr2": scr2, "stats": stats,
                 "stats_bf": stats_bf, "ps2": ps2, "ps1": ps1}

        h_t = hpool.tile([128, C, NTOK], F32)
        for c in range(C):
            nc.sync.dma_start(h_t[:, c, 0:512], h0_d[:].rearrange(
                "p (c t) -> p c t", t=NTOK)[:, c, 0:512])

        # layer-0 weights hoisted so wv/wq/wk transfer before h0's second
        # token-tile and long before the consts
        def load_w(nm, d_t, pool, kc, m, i=0):
            t = pool.tile([128, kc, m], BF16, tag=nm)
            nc.sync.dma_start(t[:], d_t[i].rearrange(
                "p (c m) -> p c m", m=m))
            return t

        w_l0 = {"wv": load_w("wv", wv_d, wqkv, C, 512),
                "wq": load_w("wq", wq_d, wqkv, C, 512),
                "wk": load_w("wk", wk_d, wqkv, C, 512)}
        for c in range(C):
            nc.sync.dma_start(h_t[:, c, 512:1024], h0_d[:].rearrange(
                "p (c t) -> p c t", t=NTOK)[:, c, 512:1024])
        w_l0["pw"] = load_w("pw", pw_d, wqkv, C, 512)
        w_l0["f1"] = load_w("f1", f1_d, wff1, C, FF)
        w_l0["f2"] = load_w("f2", f2_d, wff2, CF, 512)

        ones_t = consts.tile([128, 128], BF16)
        nc.gpsimd.memset(ones_t[:], 1.0)
        eps2_t = consts.tile([128, 1], F32)
        nc.gpsimd.memset(eps2_t[:], float(E) * float(E) * EPS)
        tri_t = consts.tile([128, 128], BF16)
        nc.sync.dma_start(tri_t[:], tri_d[:])
        pb_t = consts.tile([128, NB * C], F32)
        nc.sync.dma_start(pb_t[:], pb_d[:])
        fb1_t = consts.tile([128, NB * CF], F32)
        nc.sync.dma_start(fb1_t[:], fb1_d[:])
        fb2_t = consts.tile([128, NB * C], F32)
        nc.sync.dma_start(fb2_t[:], fb2_d[:])
        ow_t = consts.tile([128, C, V], BF16)
        nc.sync.dma_start(ow_t[:], ow_d[:].rearrange("p (c v) -> p c v", v=V))
        ob_t = consts.tile([V, 1], F32)
        nc.sync.dma_start(ob_t[:], ob_d[:])
        ncs_t = consts.tile([V, 1], F32)
        nc.sync.dma_start(ncs_t[:], ncs_d[:])
        lng_t = lnb_t = None
        if ln_general_params:
            lng_t = consts.tile([128, 2 * NB + 1, C], F32)
            nc.sync.dma_start(lng_t[:], lng_d[:].rearrange(
                "p (l c) -> p l c", c=C))
            lnb_t = consts.tile([128, 2 * NB + 1, C], F32)
            nc.sync.dma_start(lnb_t[:], lnb_d[:].rearrange(
                "p (l c) -> p l c", c=C))

        # V tile: per key-block slot: [ones|v_h0|v_h1] x HP + trailing ones.
        # AV matmuls use a 128-wide lhsT = [ones|v_h0] (h2=0) or
        # [v_h1|ones-of-next-block] (h2=1) so the otherwise-idle half of the
        # PE array emits the softmax denominator (broadcast 64x) in the same
        # pass -- no denominator matmuls at all. h0's o lands on rows 64:128
        # and h1's on rows 0:64; the proj weights are permuted host-side to
        # match. v_h0/v_h1 are adjacent so the V copy is one strided copy
        # with 128-wide runs per half.
        vt_t = vt.tile([128, SEQ * NJ, HP * 192 + 64], BF16, tag="vt")
        for hp0 in range(HP + 1):
            nc.gpsimd.memset(
                vt_t[:, :, hp0 * 192:hp0 * 192 + 64], 1.0)

        def ln_params(idx):
            if ln_general_params and not ln_trivial[idx]:
                return lng_t[:, idx, :], lnb_t[:, idx, :], False
            return None, None, True

        lnf_stats = [None, None]
        for i in range(nb_run):
            # ---- this layer's weights (wv first: V is consumed first) ----
            if i == 0:
                wv_t, wq_t, wk_t = w_l0["wv"], w_l0["wq"], w_l0["wk"]
                pw_t, f1_t, f2_t = w_l0["pw"], w_l0["f1"], w_l0["f2"]
            else:
                wv_t = load_w("wv", wv_d, wqkv, C, 512, i)
                wq_t = load_w("wq", wq_d, wqkv, C, 512, i)
                wk_t = load_w("wk", wk_d, wqkv, C, 512, i)
                pw_t = load_w("pw", pw_d, wqkv, C, 512, i)
                f1_t = load_w("f1", f1_d, wff1, C, FF, i)
                f2_t = load_w("f2", f2_d, wff2, CF, 512, i)

            # ---- V projection, token-major: vT[tk, hd*64+d] ----
            def emit_vpair(jgp):
                vp = psA.tile([128, 2, 512], F32, tag="psA")
                for half in range(2):
                    jg = 2 * jgp + half
                    for c in range(C):
                        nc.tensor.matmul(vp[:, half],
                                         xn[:, c, jg * 128:(jg + 1) * 128],
                                         wv_t[:, c, :],
                                         start=(c == 0), stop=(c == C - 1))
                for half in range(2):
                    jg = 2 * jgp + half
                    src = vp[:, half, :].rearrange("p (h x) -> p h x", x=128)
                    dst = vt_t[:, jg, 0:HP * 192].rearrange(
                        "p (h x) -> p h x", x=192)
                    nc.scalar.copy(dst[:, :, 64:192], src[:])

            # ---- LN1 (layer 0: emitted here, with the tt0 V-pairs between
            #      the two token-tiles so stats(tt1) can wait on its h0 DMA
            #      without idling the PE; others peeled into the previous
            #      layer's FFN emission) ----
            # tt1 V-pairs deferred into the hp loop so the layer-boundary
            # peel-LN chain gets matmul cover that does not depend on
            # apply(tt1).
            if i == 0:
                g_ap, b_ap, triv = ln_params(0)
                ln1 = _alloc_ln(pools)
                _emit_ln_tt(nc, pools, h_t, ones_t, eps2_t, g_ap, b_ap,
                            triv, 0, *ln1)
                xn = ln1[2]
                emit_vpair(0)
                emit_vpair(1)
                _emit_ln_tt(nc, pools, h_t, ones_t, eps2_t, g_ap, b_ap,
                            triv, 1, *ln1)
            else:
                xn = xn_next
                emit_vpair(0)
                emit_vpair(1)

            o_t = osb.tile([128, C, NTOK], BF16, tag="o")

            def emit_den_o(s, hp, p_t):
                base = s * T
                # h2=0: lhsT [ones|v_h0] -> rows 0:64 = den, 64:128 = o
                # h2=1: lhsT [v_h1|ones'] -> rows 0:64 = o, 64:128 = den
                ops = []
                for h2 in range(2):
                    vb = hp * 192 + 128 * h2
                    op = ps1.tile([128, 512], F32, tag="ps1")
                    for j in range(NJ):
                        off = j * 128
                        njw = T - off
                        nc.tensor.matmul(
                            op[:, off:T],
                            vt_t[:, s * NJ + j, vb:vb + 128],
                            p_t[:, h2, POFF[j]:POFF[j] + njw],
                            start=(j == 0), stop=(j == NJ - 1))
                    ops.append(op)
                opA, opB = ops
                # reciprocals must run at base partition 0; cross-half copies
                # (DVE bank0 -> any half) align each den with its numerator
                rdA0 = stats.tile([128, 512], F32, tag="stats")
                nc.vector.reciprocal_approx_fast(out=rdA0[0:64, :],
                                                 in_=opA[0:64, :])
                rdA = stats.tile([128, 512], F32, tag="stats")
                nc.vector.tensor_copy(rdA[64:128, :], rdA0[0:64, :])
                dB = stats.tile([128, 512], F32, tag="stats")
                nc.vector.tensor_copy(dB[0:64, :], opB[64:128, :])
                rdB = stats.tile([128, 512], F32, tag="stats")
                nc.vector.reciprocal_approx_fast(out=rdB[0:64, :],
                                                 in_=dB[0:64, :])
                nc.vector.tensor_tensor(
                    o_t[64:128, hp, base:base + T], opA[64:128, 0:T],
                    rdA[64:128, :], OP.mult)
                nc.vector.tensor_tensor(
                    o_t[0:64, hp, base:base + T], opB[0:64, 0:T],
                    rdB[0:64, :], OP.mult)

            pending = None
            for hp in range(HP):
                msl = slice(hp * 128, (hp + 1) * 128)
                q_t = qk.tile([128, NTOK], BF16, tag="q")
                k_t = qk.tile([128, NTOK], BF16, tag="k")
                for tt in range(2):
                    sl = slice(tt * 512, tt * 512 + 512)
                    qp = ps1.tile([128, 512], F32, tag="ps1")
                    kp = ps1.tile([128, 512], F32, tag="ps1")
                    for c in range(C):
                        nc.tensor.matmul(qp[:], wq_t[:, c, msl],
                                         xn[:, c, sl],
                                         start=(c == 0), stop=(c == C - 1))
                        nc.tensor.matmul(kp[:], wk_t[:, c, msl],
                                         xn[:, c, sl],
                                         start=(c == 0), stop=(c == C - 1))
                    nc.vector.tensor_copy(q_t[:, sl], qp[:])
                    nc.scalar.copy(k_t[:, sl], kp[:])

                for s in range(SEQ):
                    base = s * T
                    p_t = pp.tile([128, 2, 1408], BF16, tag="p")
                    for h2 in range(2):
                        dsl = slice(h2 * 64, h2 * 64 + 64)
                        sA = psA.tile([128, 1024], F32, tag="psA")
                        sB = ps1.tile([128, 512], F32, tag="ps1")
                        locs = [sA[:, 0:512], sA[:, 512:896],
                                sB[:, 0:256], sB[:, 256:384]]
                        for j in range(NJ):
                            off = j * 128
                            nc.tensor.matmul(
                                locs[j],
                                k_t[dsl, base + off:base + off + 128],
                                q_t[dsl, base + off:base + T],
                                start=True, stop=True)
                        nc.scalar.activation(
                            p_t[:, h2, 0:896], sA[:, 0:896], AF.Exp,
                            scale=SCALE)
                        nc.scalar.activation(
                            p_t[:, h2, 896:1280], sB[:, 0:384], AF.Exp,
                            scale=SCALE)
                    m01 = p_t[:, :, 0:1024].rearrange(
                        "p h (j c) -> p h j c", c=512)[:, :, :, 0:128]
                    m23 = p_t[:, :, 896:1408].rearrange(
                        "p h (j c) -> p h j c", c=256)[:, :, :, 0:128]
                    trib = tri_t[:, None, None, :].to_broadcast(
                        (128, 2, 2, 128))
                    nc.vector.tensor_tensor(m01, m01, trib, OP.mult)
                    nc.vector.tensor_tensor(m23, m23, trib, OP.mult)
                    if pending is not None:
                        emit_den_o(*pending)
                    pending = (s, hp, p_t)
                    if hp == 0 and s == 0:
                        emit_vpair(2)
                        emit_vpair(3)

            # ---- attention out projection + residual, interleaved with
            #      LN2 so proj(tt1)'s matmuls cover LN2(tt0)'s chain ----
            def emit_proj(tt):
                sl = slice(tt * 512, tt * 512 + 512)
                for mc in range(C):
                    pj = ps1.tile([128, 512], F32, tag="ps1")
                    for c in range(C):
                        nc.tensor.matmul(pj[:],
                                         pw_t[:, c, mc * 128:(mc + 1) * 128],
                                         o_t[:, c, sl],
                                         start=(c == 0), stop=(c == C - 1))
                    nc.vector.scalar_tensor_tensor(
                        out=h_t[:, mc, sl], in0=pj[:],
                        scalar=pb_t[:, i * C + mc:i * C + mc + 1],
                        in1=h_t[:, mc, sl], op0=OP.add, op1=OP.add)

            g_ap, b_ap, triv = ln_params(2 * i + 1)
            ln2 = _alloc_ln(pools)
            emit_proj(0)
            # flush the last AV after proj(0): its matmuls cover the LN2(tt0)
            # scalar/vector chain (proj(0) only needs o_t token-tile 0)
            emit_den_o(*pending)
            _emit_ln_tt(nc, pools, h_t, ones_t, eps2_t, g_ap, b_ap,
                        triv, 0, *ln2)
            emit_proj(1)
            _emit_ln_tt(nc, pools, h_t, ones_t, eps2_t, g_ap, b_ap,
                        triv, 1, *ln2)
            xn2 = ln2[2]

            for tt in range(2):
                sl = slice(tt * 512, tt * 512 + 512)
                fa = ffa.tile([128, CF, 512], BF16, tag="fa")
                for mfp in range(CF // 2):
                    fp = psA.tile([128, 2, 512], F32, tag="psA")
                    for half in range(2):
                        mf = 2 * mfp + half
                        for c in range(C):
                            nc.tensor.matmul(
                                fp[:, half],
                                f1_t[:, c, mf * 128:(mf + 1) * 128],
                                xn2[:, c, sl],
                                start=(c == 0), stop=(c == C - 1))
                    if bias_trivial[i]:
                        nc.scalar.activation(
                            fa[:, 2 * mfp:2 * mfp + 2, :], fp[:], AF.Relu)
                    else:
                        for half in range(2):
                            mf = 2 * mfp + half
                            nc.scalar.activation(
                                fa[:, mf, :], fp[:, half], AF.Relu,
                                bias=fb1_t[:, i * CF + mf:i * CF + mf + 1])
                for mc in range(C):
                    f2p = ps1.tile([128, 512], F32, tag="ps1")
                    for c16 in range(CF):
                        nc.tensor.matmul(f2p[:],
                                         f2_t[:, c16, mc * 128:(mc + 1) * 128],
                                         fa[:, c16, :],
                                         start=(c16 == 0),
                                         stop=(c16 == CF - 1))
                    nc.vector.scalar_tensor_tensor(
                        out=h_t[:, mc, sl], in0=f2p[:],
                        scalar=fb2_t[:, i * C + mc:i * C + mc + 1],
                        in1=h_t[:, mc, sl], op0=OP.add, op1=OP.add)
                # peel next layer's LN1(tt) (or the final LN on the last
                # layer) here so its scalar/vector chain hides behind the
                # other token-tile's FFN matmuls
                if i + 1 < nb_run:
                    if tt == 0:
                        ln_next = _alloc_ln(pools)
                    g_ap, b_ap, triv = ln_params(2 * (i + 1))
                    _emit_ln_tt(nc, pools, h_t, ones_t, eps2_t, g_ap, b_ap,
                                triv, tt, *ln_next)
                    if tt == 1:
                        xn_next = ln_next[2]
                elif nb_run == NB:
                    # final LN: stats only; the normalize affine is folded
                    # into the logits output: logits = r*(P - s1*cs/E) (+ob)
                    # where P = hb@ow. t1 = P - s1*cs/E is computed as soon
                    # as the stats land so only r (std->recip chain) remains
                    # on the tail.
                    def emit_logits_pre(ltt):
                        sl2 = slice(ltt * 512, ltt * 512 + 512)
                        s1c = lnf_stats[ltt][1]
                        lg = ps1.tile([V, 512], F32, tag="ps1")
                        for c in range(C):
                            nc.tensor.matmul(lg[:], ow_t[:, c, :],
                                             ln_next[0][:, c, sl2],
                                             start=(c == 0),
                                             stop=(c == C - 1))
                        t1 = stats_bf.tile([128, 512], BF16, tag="lgt")
                        nc.vector.scalar_tensor_tensor(
                            out=t1[0:V, :], in0=s1c[0:V, :], scalar=ncs_t[:],
                            in1=lg[:], op0=OP.mult, op1=OP.add)
                        lnf_stats[ltt] = (lnf_stats[ltt][0], t1)
                    if tt == 0:
                        ln_next = _alloc_ln(pools)
                    else:
                        emit_logits_pre(0)
                    r_bf, s1 = _emit_ln_tt(
                        nc, pools, h_t, ones_t, eps2_t, None, None, True,
                        tt, *ln_next, apply=False)
                    s1c = stats.tile([128, 512], F32, tag="stats")
                    nc.scalar.copy(s1c[0:V, :], s1[0:V, :])
                    lnf_stats[tt] = (r_bf, s1c)
                    if tt == 1:
                        emit_logits_pre(1)

        # ---- logits tail: out = t1 * r (+ ob) ----
        if nb_run == NB:
            for tt in range(2):
                sl = slice(tt * 512, tt * 512 + 512)
                r_bf, t1 = lnf_stats[tt]
                lgs = stats.tile([128, 512], F32, tag="stats")
                nc.vector.tensor_tensor(lgs[0:V, :], t1[0:V, :],
                                        r_bf[0:V, :], OP.mult)
                if not ob_trivial:
                    nc.vector.tensor_scalar_add(lgs[0:V, :], lgs[0:V, :],
                                                ob_t[:])
                nc.sync.dma_start(out_d[:, sl], lgs[0:V, :])
        else:
            xnf = _emit_ln(nc, pools, h_t, ones_t, eps2_t, None, None, True)
            for tt in range(2):
                sl = slice(tt * 512, tt * 512 + 512)
                lg = ps1.tile([V, 512], F32, tag="ps1")
                lgs = stats.tile([128, 512], F32, tag="stats")
                for c in range(C):
                    nc.tensor.matmul(lg[:], ow_t[:, c, :], xnf[:, c, sl],
                                     start=(c == 0), stop=(c == C - 1))
                nc.vector.tensor_scalar_add(lgs[0:V, :], lg[:], ob_t[:])
                nc.sync.dma_start(out_d[:, sl], lgs[0:V, :])

    nc.finalize()
    return nc


def prepare_inputs(inputs):
    """Host-side preprocessing: embedding gather, weight layout + bf16 cast.
    Returns (shared_map, per_core_h0_list, ln_trivial, bias_trivial)."""
    f32 = np.float32
    bf16 = ml_dtypes.bfloat16
    x = np.asarray(inputs["x"]).astype(np.int64)
    emb = np.asarray(inputs["emb"], dtype=f32)
    pos = np.asarray(inputs["pos"], dtype=f32)

    positions = np.minimum(np.arange(T), L - 1)
    h0 = emb[x] + pos[positions][None, :, :]      # [B, T, E] fp32

    def to_dev_lhst(mat, kchunks, mcols):
        m = np.ascontiguousarray(mat.astype(bf16))
        return m.reshape(kchunks, 128, mcols).transpose(1, 0, 2).reshape(
            128, kchunks * mcols)

    wq = np.asarray(inputs["wq"], dtype=f32)
    wk = np.asarray(inputs["wk"], dtype=f32)
    wv = np.asarray(inputs["wv"], dtype=f32)
    pw = np.asarray(inputs["proj_w"], dtype=f32)
    f1 = np.asarray(inputs["ff_w1"], dtype=f32)
    f2 = np.asarray(inputs["ff_w2"], dtype=f32)

    wq_dev = np.stack([to_dev_lhst(wq[i].transpose(1, 0, 2).reshape(E, NH * HS),
                                   C, 512) for i in range(NB)])
    wk_dev = np.stack([to_dev_lhst(wk[i].transpose(1, 0, 2).reshape(E, NH * HS),
                                   C, 512) for i in range(NB)])
    wv_dev = np.stack([to_dev_lhst(wv[i].transpose(1, 0, 2).reshape(E, NH * HS),
                                   C, 512) for i in range(NB)])
    # o_t holds [h1|h0] per head-pair chunk (see vt layout in build_program):
    # permute proj_w rows to match
    pw_perm = np.concatenate([
        np.concatenate([np.arange(hp * 128 + 64, hp * 128 + 128),
                        np.arange(hp * 128, hp * 128 + 64)])
        for hp in range(HP)])
    pw_dev = np.stack([to_dev_lhst(pw[i][pw_perm], C, 512)
                       for i in range(NB)])
    f1_dev = np.stack([to_dev_lhst(f1[i], C, FF) for i in range(NB)])
    f2_dev = np.stack([to_dev_lhst(f2[i], CF, 512) for i in range(NB)])

    def vec_dev(v, chunks):
        return np.ascontiguousarray(v.astype(f32).reshape(chunks, 128).T)

    pb_dev = np.concatenate([vec_dev(np.asarray(inputs["proj_b"][i]), C)
                             for i in range(NB)], axis=1)
    fb1_dev = np.concatenate([vec_dev(np.asarray(inputs["ff_b1"][i]), CF)
                              for i in range(NB)], axis=1)
    fb2_dev = np.concatenate([vec_dev(np.asarray(inputs["ff_b2"][i]), C)
                              for i in range(NB)], axis=1)
    # fold the final-LN affine into the logits head:
    # logits = LNstat(h)*g@W/T + b@W/T + ob/T = r*(h@W') - mu*colsum(W') + ob'
    lnf_g_v = np.asarray(inputs["lnf_g"], dtype=f32)
    lnf_b_v = np.asarray(inputs["lnf_b"], dtype=f32)
    out_w_v = np.asarray(inputs["out_w"], dtype=f32)
    ow_eff = (lnf_g_v[:, None] * out_w_v) / TEMP
    ow_dev = to_dev_lhst(ow_eff, C, V)
    ncs_dev = np.ascontiguousarray((-ow_eff.sum(axis=0) / E).reshape(V, 1))
    ob_eff = (np.asarray(inputs["out_b"], dtype=f32)
              + lnf_b_v @ out_w_v) / TEMP
    ob_dev = np.ascontiguousarray(ob_eff.reshape(V, 1))
    ob_trivial = bool(np.all(ob_eff == 0.0))
    tri_dev = np.triu(np.ones((128, 128), dtype=f32)).astype(bf16)

    gs, bs, ln_trivial = [], [], []
    for i in range(NB):
        for nm_g, nm_b in (("ln1_g", "ln1_b"), ("ln2_g", "ln2_b")):
            g = np.asarray(inputs[nm_g][i], dtype=f32)
            b = np.asarray(inputs[nm_b][i], dtype=f32)
            gs.append(vec_dev(g, C))
            bs.append(vec_dev(b, C))
            ln_trivial.append(bool(np.all(g == 1.0) and np.all(b == 0.0)))
    g = np.asarray(inputs["lnf_g"], dtype=f32)
    b = np.asarray(inputs["lnf_b"], dtype=f32)
    gs.append(vec_dev(g, C))
    bs.append(vec_dev(b, C))
    ln_trivial.append(bool(np.all(g == 1.0) and np.all(b == 0.0)))
    lng_dev = np.concatenate(gs, axis=1)
    lnb_dev = np.concatenate(bs, axis=1)

    ln_trivial.append(ob_trivial)   # threaded through to build_program

    shared = {
        "wq": wq_dev, "wk": wk_dev, "wv": wv_dev, "pw": pw_dev,
        "f1": f1_dev, "f2": f2_dev, "pb": pb_dev, "fb1": fb1_dev,
        "fb2": fb2_dev, "ow": ow_dev, "ob": ob_dev, "ncs": ncs_dev,
        "tri": tri_dev, "lng": lng_dev, "lnb": lnb_dev,
    }

    h0_cores = []
    for core in range(NCORES):
        hh = h0[SEQ * core:SEQ * core + SEQ]          # [SEQ, T, E]
        hT = hh.transpose(2, 0, 1).reshape(E, NTOK)   # [E, NTOK]
        h0_cores.append(np.ascontiguousarray(
            hT.reshape(C, 128, NTOK).transpose(1, 0, 2).reshape(
                128, C * NTOK)))
    bias_trivial = []
    for i in range(NB):
        bias_trivial.append(bool(
            np.all(np.asarray(inputs["proj_b"][i]) == 0.0)
            and np.all(np.asarray(inputs["ff_b1"][i]) == 0.0)
            and np.all(np.asarray(inputs["ff_b2"][i]) == 0.0)))
    return shared, h0_cores, ln_trivial, bias_trivial


def assemble_output(core_logits):
    """core_logits: list of [V, NTOK] fp32 -> [B, T, V]."""
    out = np.empty((B, T, V), np.float32)
    for core in range(NCORES):
        lg = core_logits[core].reshape(V, SEQ, T)
        out[SEQ * core:SEQ * core + SEQ] = lg.transpose(1, 2, 0)
    return out


def get_program(ln_trivial, bias_trivial):
    key = (tuple(ln_trivial), tuple(bias_trivial))
    if key not in _PROGRAM_CACHE:
        lt = list(key[0])
        _PROGRAM_CACHE[key] = build_program(
            lt[:2 * NB + 1], list(key[1]), ob_trivial=bool(lt[-1]))
    return _PROGRAM_CACHE[key]


def reset_device():
    """Recover a wedged accelerator (axon session reset). Best-effort."""
    try:
        import ctypes
        import jax
        jax.devices()
        lib = ctypes.CDLL('/opt/axon/libaxon_pjrt.so')
        lib.axon_reset.restype = ctypes.c_int64
        lib.axon_reset()
    except Exception:
        pass


def kernel(**inputs):
    from concourse.bass_utils import run_bass_kernel_spmd
    shared, h0_cores, ln_trivial, bias_trivial = prepare_inputs(inputs)
    nc = get_program(ln_trivial, bias_trivial)
    in_maps = [dict(shared, h0=h0_cores[c]) for c in range(NCORES)]
    try:
        res = run_bass_kernel_spmd(nc, in_maps, core_ids=list(range(NCORES)))
    except Exception:
        # A previous (profiled) session can leave the device wedged; reset
        # the axon session and retry once.
        reset_device()
        res = run_bass_kernel_spmd(nc, in_maps, core_ids=list(range(NCORES)))
    return assemble_output([res.results[c]["logits"] for c in range(NCORES)])



# revision 85
# speedup vs baseline: 1.0082x; 1.0033x over previous
"""Trainium2 Bass kernel for nn_AutoregressiveArithmeticTransformer.

6-layer dense transformer: B=16, T=512, E=512, NH=8 heads x HS=64, FF=2048,
V=16, causal attention, pre-LN, learned abacus embedding, logits / 0.8.

Strategy: data-parallel over batch across 8 NeuronCores (2 sequences per
core, no collectives). Activations live feature-major in SBUF
([E-partitions, tokens]); weights are streamed per-layer in bf16; all
matmuls run in bf16 with fp32 PSUM accumulation; the residual stream stays
fp32. LayerNorm statistics are computed with ones-matmuls on the PE;
attention scores are computed transposed ([tk, tq]) so the softmax
denominator is also a ones-matmul; V is produced token-major directly so
no transposes are ever needed.

All ops are token-tile (512) granular so the two sequences per core form
independent dependency streams the Tile scheduler can interleave.
"""

import numpy as np
import ml_dtypes

import concourse.bacc as bacc
import concourse.tile as tile
from concourse import mybir

F32 = mybir.dt.float32
F32R = mybir.dt.float32r
BF16 = mybir.dt.bfloat16
AF = mybir.ActivationFunctionType
OP = mybir.AluOpType

# Model constants (hardcoded per contest contract)
V, E, NH, HS, FF, NB, L = 16, 512, 8, 64, 2048, 6, 512
B, T = 16, 512
TEMP = 1.0 * 0.8
EPS = 1e-5
SCALE = HS ** -0.5  # 0.125

NCORES = 8
SEQ = 2              # sequences per core
NTOK = SEQ * T       # 1024 tokens per core
C = E // 128         # 4 E-chunks
CF = FF // 128       # 16 FF-chunks
HP = NH // 2         # 4 head-pairs
NJ = T // 128        # 4 tk chunks per sequence
NJW = [T - 128 * j for j in range(NJ)]          # [512, 384, 256, 128]
POFF = [0, 512, 896, 1152]                      # compact score offsets

_PROGRAM_CACHE = {}


def _emit_ln_tt(nc, pools, h_t, ones_t, eps2_t, g_ap, b_ap, trivial, tt,
                hb, sq, xn, apply=True):
    """One token-tile of LayerNorm into caller-allocated hb/sq/xn tiles.
    With apply=False the normalize is not applied; returns (r_bf, z_bf) so
    the caller can fold the affine into a downstream matmul output."""
    stats, stats_bf = pools["stats"], pools["stats_bf"]
    ps1 = pools["ps1"]
    sl = slice(tt * 512, tt * 512 + 512)
    s1 = ps1.tile([128, 512], F32, tag="ps1", name="s1")
    s2 = ps1.tile([128, 512], F32, tag="ps1", name="s2")
    for c in range(C):
        nc.scalar.copy(hb[:, c, sl], h_t[:, c, sl])
        sq = pools["scr2"].tile([128, 512], BF16, tag="sq", name="sq")
        nc.vector.tensor_tensor(sq[:], hb[:, c, sl], hb[:, c, sl],
                                OP.mult)
        nc.tensor.matmul(s1[:], ones_t[:], hb[:, c, sl],
                         start=(c == 0), stop=(c == C - 1))
        nc.tensor.matmul(s2[:], ones_t[:], sq[:],
                         start=(c == 0), stop=(c == C - 1))
    msq = stats.tile([128, 512], F32, tag="stats")
    nc.scalar.square(msq[:], s1[:])
    var = stats.tile([128, 512], F32, tag="stats")
    nc.vector.scalar_tensor_tensor(out=var[:], in0=s2[:], scalar=float(E),
                                   in1=msq[:], op0=OP.mult, op1=OP.subtract)
    std = stats.tile([128, 512], F32, tag="stats")
    nc.scalar.activation(std[:], var[:], AF.Sqrt, bias=eps2_t[:])
    rc = stats.tile([128, 512], F32, tag="stats")
    nc.vector.reciprocal_approx_fast(out=rc[:], in_=std[:])
    r_bf = stats_bf.tile([128, 512], BF16, tag="r_bf")
    nc.scalar.mul(r_bf[:], rc[:], float(E))
    if not apply:
        return r_bf, s1
    z_bf = stats_bf.tile([128, 512], BF16, tag="z_bf")
    nc.vector.tensor_tensor(z_bf[:], s1[:], rc[:], OP.mult)
    for c in range(C):
        nc.vector.tensor_tensor(xn[:, c, sl], hb[:, c, sl], r_bf[:], OP.mult)
        nc.vector.tensor_tensor(xn[:, c, sl], xn[:, c, sl], z_bf[:],
                                OP.subtract)
        if not trivial:
            nc.vector.tensor_scalar(out=xn[:, c, sl], in0=xn[:, c, sl],
                                    scalar1=g_ap[:, c:c + 1],
                                    scalar2=b_ap[:, c:c + 1],
                                    op0=OP.mult, op1=OP.add)
    return None


def _alloc_ln(pools):
    hb = pools["scr"].tile([128, C, NTOK], BF16, tag="scratch", name="hb")
    xn = pools["scr"].tile([128, C, NTOK], BF16, tag="scratch", name="xnt")
    return hb, None, xn


def _emit_ln(nc, pools, h_t, ones_t, eps2_t, g_ap, b_ap, trivial):
    hb, sq, xn = _alloc_ln(pools)
    for tt in range(2):
        _emit_ln_tt(nc, pools, h_t, ones_t, eps2_t, g_ap, b_ap, trivial, tt,
                    hb, sq, xn)
    return xn


def build_program(ln_trivial, bias_trivial=None, nb_run=NB,
                  ln_general_params=True, ob_trivial=True):
    """Build the Bass program. ln_trivial: list of NB*2+1 bools (ln1/ln2 per
    layer then lnf) -- when True the g/b application op is skipped."""
    if bias_trivial is None:
        bias_trivial = [False] * NB
    nc = bacc.Bacc(None, target_bir_lowering=False)

    h0_d = nc.dram_tensor("h0", [128, C * NTOK], F32, kind="ExternalInput")
    wq_d = nc.dram_tensor("wq", [NB, 128, C * 512], BF16, kind="ExternalInput")
    wk_d = nc.dram_tensor("wk", [NB, 128, C * 512], BF16, kind="ExternalInput")
    wv_d = nc.dram_tensor("wv", [NB, 128, C * 512], BF16, kind="ExternalInput")
    pw_d = nc.dram_tensor("pw", [NB, 128, C * 512], BF16, kind="ExternalInput")
    f1_d = nc.dram_tensor("f1", [NB, 128, C * FF], BF16, kind="ExternalInput")
    f2_d = nc.dram_tensor("f2", [NB, 128, CF * 512], BF16, kind="ExternalInput")
    pb_d = nc.dram_tensor("pb", [128, NB * C], F32, kind="ExternalInput")
    fb1_d = nc.dram_tensor("fb1", [128, NB * CF], F32, kind="ExternalInput")
    fb2_d = nc.dram_tensor("fb2", [128, NB * C], F32, kind="ExternalInput")
    ow_d = nc.dram_tensor("ow", [128, C * V], BF16, kind="ExternalInput")
    ob_d = nc.dram_tensor("ob", [V, 1], F32, kind="ExternalInput")
    ncs_d = nc.dram_tensor("ncs", [V, 1], F32, kind="ExternalInput")
    tri_d = nc.dram_tensor("tri", [128, 128], BF16, kind="ExternalInput")
    lng_d = lnb_d = None
    if ln_general_params:
        lng_d = nc.dram_tensor("lng", [128, (2 * NB + 1) * C], F32,
                               kind="ExternalInput")
        lnb_d = nc.dram_tensor("lnb", [128, (2 * NB + 1) * C], F32,
                               kind="ExternalInput")
    out_d = nc.dram_tensor("logits", [V, NTOK], F32, kind="ExternalOutput")

    from contextlib import ExitStack
    with ExitStack() as ctx:
        tc = ctx.enter_context(tile.TileContext(nc))
        consts = ctx.enter_context(tc.tile_pool(name="consts", bufs=1))
        hpool = ctx.enter_context(tc.tile_pool(name="hpool", bufs=1))
        wqkv = ctx.enter_context(tc.tile_pool(name="wqkv", bufs=1))
        wff1 = ctx.enter_context(tc.tile_pool(name="wff1", bufs=2))
        wff2 = ctx.enter_context(tc.tile_pool(name="wff2", bufs=1))
        scr = ctx.enter_context(tc.tile_pool(name="scr", bufs=4))
        scr2 = ctx.enter_context(tc.tile_pool(name="scr2", bufs=2))
        qk = ctx.enter_context(tc.tile_pool(name="qk", bufs=2))
        vt = ctx.enter_context(tc.tile_pool(name="vt", bufs=1))
        pp = ctx.enter_context(tc.tile_pool(name="pp", bufs=2))
        osb = ctx.enter_context(tc.tile_pool(name="osb", bufs=1))
        ffa = ctx.enter_context(tc.tile_pool(name="ffa", bufs=2))
        stats = ctx.enter_context(tc.tile_pool(name="stats", bufs=7))
        stats_bf = ctx.enter_context(tc.tile_pool(name="stats_bf", bufs=2))
        ps1 = ctx.enter_context(tc.tile_pool(name="ps1", bufs=4, space="PSUM"))
        psA = ctx.enter_context(tc.tile_pool(name="psA", bufs=2, space="PSUM"))
        ps2 = ps1

        pools = {"scr": scr, "scr2": scr2, "stats": stats,
                 "stats_bf": stats_bf, "ps2": ps2, "ps1": ps1}

        h_t = hpool.tile([128, C, NTOK], F32)
        for c in range(C):
            nc.sync.dma_start(h_t[:, c, 0:512], h0_d[:].rearrange(
                "p (c t) -> p c t", t=NTOK)[:, c, 0:512])

        # layer-0 weights hoisted so wv/wq/wk transfer before h0's second
        # token-tile and long before the consts
        def load_w(nm, d_t, pool, kc, m, i=0):
            t = pool.tile([128, kc, m], BF16, tag=nm)
            nc.sync.dma_start(t[:], d_t[i].rearrange(
                "p (c m) -> p c m", m=m))
            return t

        w_l0 = {"wv": load_w("wv", wv_d, wqkv, C, 512),
                "wq": load_w("wq", wq_d, wqkv, C, 512),
                "wk": load_w("wk", wk_d, wqkv, C, 512)}
        for c in range(C):
            nc.sync.dma_start(h_t[:, c, 512:1024], h0_d[:].rearrange(
                "p (c t) -> p c t", t=NTOK)[:, c, 512:1024])
        w_l0["pw"] = load_w("pw", pw_d, wqkv, C, 512)
        w_l0["f1"] = load_w("f1", f1_d, wff1, C, FF)
        w_l0["f2"] = load_w("f2", f2_d, wff2, CF, 512)

        ones_t = consts.tile([128, 128], BF16)
        nc.gpsimd.memset(ones_t[:], 1.0)
        eps2_t = consts.tile([128, 1], F32)
        nc.gpsimd.memset(eps2_t[:], float(E) * float(E) * EPS)
        tri_t = consts.tile([128, 128], BF16)
        nc.sync.dma_start(tri_t[:], tri_d[:])
        pb_t = consts.tile([128, NB * C], F32)
        nc.sync.dma_start(pb_t[:], pb_d[:])
        fb1_t = consts.tile([128, NB * CF], F32)
        nc.sync.dma_start(fb1_t[:], fb1_d[:])
        fb2_t = consts.tile([128, NB * C], F32)
        nc.sync.dma_start(fb2_t[:], fb2_d[:])
        ow_t = consts.tile([128, C, V], BF16)
        nc.sync.dma_start(ow_t[:], ow_d[:].rearrange("p (c v) -> p c v", v=V))
        ob_t = consts.tile([V, 1], F32)
        nc.sync.dma_start(ob_t[:], ob_d[:])
        ncs_t = consts.tile([V, 1], F32)
        nc.sync.dma_start(ncs_t[:], ncs_d[:])
        lng_t = lnb_t = None
        if ln_general_params:
            lng_t = consts.tile([128, 2 * NB + 1, C], F32)
            nc.sync.dma_start(lng_t[:], lng_d[:].rearrange(
                "p (l c) -> p l c", c=C))
            lnb_t = consts.tile([128, 2 * NB + 1, C], F32)
            nc.sync.dma_start(lnb_t[:], lnb_d[:].rearrange(
                "p (l c) -> p l c", c=C))

        # V tile: per key-block slot: [ones|v_h0|v_h1] x HP + trailing ones.
        # AV matmuls use a 128-wide lhsT = [ones|v_h0] (h2=0) or
        # [v_h1|ones-of-next-block] (h2=1) so the otherwise-idle half of the
        # PE array emits the softmax denominator (broadcast 64x) in the same
        # pass -- no denominator matmuls at all. h0's o lands on rows 64:128
        # and h1's on rows 0:64; the proj weights are permuted host-side to
        # match. v_h0/v_h1 are adjacent so the V copy is one strided copy
        # with 128-wide runs per half.
        vt_t = vt.tile([128, SEQ * NJ, HP * 192 + 64], BF16, tag="vt")
        for hp0 in range(HP + 1):
            nc.gpsimd.memset(
                vt_t[:, :, hp0 * 192:hp0 * 192 + 64], 1.0)

        def ln_params(idx):
            if ln_general_params and not ln_trivial[idx]:
                return lng_t[:, idx, :], lnb_t[:, idx, :], False
            return None, None, True

        lnf_stats = [None, None]
        for i in range(nb_run):
            # ---- this layer's weights (wv first: V is consumed first) ----
            if i == 0:
                wv_t, wq_t, wk_t = w_l0["wv"], w_l0["wq"], w_l0["wk"]
                pw_t, f1_t, f2_t = w_l0["pw"], w_l0["f1"], w_l0["f2"]
            else:
                wv_t = load_w("wv", wv_d, wqkv, C, 512, i)
                wq_t = load_w("wq", wq_d, wqkv, C, 512, i)
                wk_t = load_w("wk", wk_d, wqkv, C, 512, i)
                pw_t = load_w("pw", pw_d, wqkv, C, 512, i)
                f1_t = load_w("f1", f1_d, wff1, C, FF, i)
                f2_t = load_w("f2", f2_d, wff2, CF, 512, i)

            # ---- V projection, token-major: vT[tk, hd*64+d] ----
            def emit_vpair(jgp):
                vp = psA.tile([128, 2, 512], F32, tag="psA")
                for half in range(2):
                    jg = 2 * jgp + half
                    for c in range(C):
                        nc.tensor.matmul(vp[:, half],
                                         xn[:, c, jg * 128:(jg + 1) * 128],
                                         wv_t[:, c, :],
                                         start=(c == 0), stop=(c == C - 1))
                for half in range(2):
                    jg = 2 * jgp + half
                    src = vp[:, half, :].rearrange("p (h x) -> p h x", x=128)
                    dst = vt_t[:, jg, 0:HP * 192].rearrange(
                        "p (h x) -> p h x", x=192)
                    nc.scalar.copy(dst[:, :, 64:192], src[:])

            # ---- LN1 (layer 0: emitted here, with the tt0 V-pairs between
            #      the two token-tiles so stats(tt1) can wait on its h0 DMA
            #      without idling the PE; others peeled into the previous
            #      layer's FFN emission) ----
            # tt1 V-pairs deferred into the hp loop so the layer-boundary
            # peel-LN chain gets matmul cover that does not depend on
            # apply(tt1).
            if i == 0:
                g_ap, b_ap, triv = ln_params(0)
                ln1 = _alloc_ln(pools)
                _emit_ln_tt(nc, pools, h_t, ones_t, eps2_t, g_ap, b_ap,
                            triv, 0, *ln1)
                xn = ln1[2]
                emit_vpair(0)
                emit_vpair(1)
                _emit_ln_tt(nc, pools, h_t, ones_t, eps2_t, g_ap, b_ap,
                            triv, 1, *ln1)
            else:
                xn = xn_next
                emit_vpair(0)
                emit_vpair(1)

            o_t = osb.tile([128, C, NTOK], BF16, tag="o")

            def emit_den_o(s, hp, p_t):
                base = s * T
                # h2=0: lhsT [ones|v_h0] -> rows 0:64 = den, 64:128 = o
                # h2=1: lhsT [v_h1|ones'] -> rows 0:64 = o, 64:128 = den
                ops = []
                for h2 in range(2):
                    vb = hp * 192 + 128 * h2
                    op = ps1.tile([128, 512], F32, tag="ps1")
                    for j in range(NJ):
                        off = j * 128
                        njw = T - off
                        nc.tensor.matmul(
                            op[:, off:T],
                            vt_t[:, s * NJ + j, vb:vb + 128],
                            p_t[:, h2, POFF[j]:POFF[j] + njw],
                            start=(j == 0), stop=(j == NJ - 1))
                    ops.append(op)
                opA, opB = ops
                # reciprocals must run at base partition 0; cross-half copies
                # (DVE bank0 -> any half) align each den with its numerator
                rdA0 = stats.tile([128, 512], F32, tag="stats")
                nc.vector.reciprocal_approx_fast(out=rdA0[0:64, :],
                                                 in_=opA[0:64, :])
                rdA = stats.tile([128, 512], F32, tag="stats")
                nc.vector.tensor_copy(rdA[64:128, :], rdA0[0:64, :])
                dB = stats.tile([128, 512], F32, tag="stats")
                nc.vector.tensor_copy(dB[0:64, :], opB[64:128, :])
                rdB = stats.tile([128, 512], F32, tag="stats")
                nc.vector.reciprocal_approx_fast(out=rdB[0:64, :],
                                                 in_=dB[0:64, :])
                nc.vector.tensor_tensor(
                    o_t[64:128, hp, base:base + T], opA[64:128, 0:T],
                    rdA[64:128, :], OP.mult)
                nc.vector.tensor_tensor(
                    o_t[0:64, hp, base:base + T], opB[0:64, 0:T],
                    rdB[0:64, :], OP.mult)

            pending = None
            for hp in range(HP):
                msl = slice(hp * 128, (hp + 1) * 128)
                q_t = qk.tile([128, NTOK], BF16, tag="q")
                k_t = qk.tile([128, NTOK], BF16, tag="k")
                for tt in range(2):
                    sl = slice(tt * 512, tt * 512 + 512)
                    qp = ps1.tile([128, 512], F32, tag="ps1")
                    kp = ps1.tile([128, 512], F32, tag="ps1")
                    for c in range(C):
                        nc.tensor.matmul(qp[:], wq_t[:, c, msl],
                                         xn[:, c, sl],
                                         start=(c == 0), stop=(c == C - 1))
                        nc.tensor.matmul(kp[:], wk_t[:, c, msl],
                                         xn[:, c, sl],
                                         start=(c == 0), stop=(c == C - 1))
                    nc.vector.tensor_copy(q_t[:, sl], qp[:])
                    nc.scalar.copy(k_t[:, sl], kp[:])

                for s in range(SEQ):
                    base = s * T
                    p_t = pp.tile([128, 2, 1408], BF16, tag="p")
                    for h2 in range(2):
                        dsl = slice(h2 * 64, h2 * 64 + 64)
                        sA = psA.tile([128, 1024], F32, tag="psA")
                        sB = ps1.tile([128, 512], F32, tag="ps1")
                        locs = [sA[:, 0:512], sA[:, 512:896],
                                sB[:, 0:256], sB[:, 256:384]]
                        for j in range(NJ):
                            off = j * 128
                            nc.tensor.matmul(
                                locs[j],
                                k_t[dsl, base + off:base + off + 128],
                                q_t[dsl, base + off:base + T],
                                start=True, stop=True)
                        nc.scalar.activation(
                            p_t[:, h2, 0:896], sA[:, 0:896], AF.Exp,
                            scale=SCALE)
                        nc.scalar.activation(
                            p_t[:, h2, 896:1280], sB[:, 0:384], AF.Exp,
                            scale=SCALE)
                    m01 = p_t[:, :, 0:1024].rearrange(
                        "p h (j c) -> p h j c", c=512)[:, :, :, 0:128]
                    m23 = p_t[:, :, 896:1408].rearrange(
                        "p h (j c) -> p h j c", c=256)[:, :, :, 0:128]
                    trib = tri_t[:, None, None, :].to_broadcast(
                        (128, 2, 2, 128))
                    nc.vector.tensor_tensor(m01, m01, trib, OP.mult)
                    nc.vector.tensor_tensor(m23, m23, trib, OP.mult)
                    if pending is not None:
                        emit_den_o(*pending)
                    pending = (s, hp, p_t)
                    if hp == 0 and s == 0:
                        emit_vpair(2)
                        emit_vpair(3)

            # ---- attention out projection + residual, interleaved with
            #      LN2 so proj(tt1)'s matmuls cover LN2(tt0)'s chain ----
            def emit_proj(tt):
                sl = slice(tt * 512, tt * 512 + 512)
                for mc in range(C):
                    pj = ps1.tile([128, 512], F32, tag="ps1")
                    for c in range(C):
                        nc.tensor.matmul(pj[:],
                                         pw_t[:, c, mc * 128:(mc + 1) * 128],
                                         o_t[:, c, sl],
                                         start=(c == 0), stop=(c == C - 1))
                    nc.vector.scalar_tensor_tensor(
                        out=h_t[:, mc, sl], in0=pj[:],
                        scalar=pb_t[:, i * C + mc:i * C + mc + 1],
                        in1=h_t[:, mc, sl], op0=OP.add, op1=OP.add)

            g_ap, b_ap, triv = ln_params(2 * i + 1)
            ln2 = _alloc_ln(pools)
            emit_proj(0)
            # flush the last AV after proj(0): its matmuls cover the LN2(tt0)
            # scalar/vector chain (proj(0) only needs o_t token-tile 0)
            emit_den_o(*pending)
            _emit_ln_tt(nc, pools, h_t, ones_t, eps2_t, g_ap, b_ap,
                        triv, 0, *ln2)
            emit_proj(1)
            _emit_ln_tt(nc, pools, h_t, ones_t, eps2_t, g_ap, b_ap,
                        triv, 1, *ln2)
            xn2 = ln2[2]

            for tt in range(2):
                sl = slice(tt * 512, tt * 512 + 512)
                fa = ffa.tile([128, CF, 512], BF16, tag="fa")
                for mfp in range(CF // 2):
                    fp = psA.tile([128, 2, 512], F32, tag="psA")
                    for half in range(2):
                        mf = 2 * mfp + half
                        for c in range(C):
                            nc.tensor.matmul(
                                fp[:, half],
                                f1_t[:, c, mf * 128:(mf + 1) * 128],
                                xn2[:, c, sl],
                                start=(c == 0), stop=(c == C - 1))
                    if bias_trivial[i]:
                        nc.scalar.activation(
                            fa[:, 2 * mfp:2 * mfp + 2, :], fp[:], AF.Relu)
                    else:
                        for half in range(2):
                            mf = 2 * mfp + half
                            nc.scalar.activation(
                                fa[:, mf, :], fp[:, half], AF.Relu,
                                bias=fb1_t[:, i * CF + mf:i * CF + mf + 1])
                for mc in range(C):
                    f2p = ps1.tile([128, 512], F32, tag="ps1")
                    for c16 in range(CF):
                        nc.tensor.matmul(f2p[:],
                                         f2_t[:, c16, mc * 128:(mc + 1) * 128],
                                         fa[:, c16, :],
                                         start=(c16 == 0),
                                         stop=(c16 == CF - 1))
                    nc.vector.scalar_tensor_tensor(
                        out=h_t[:, mc, sl], in0=f2p[:],
                        scalar=fb2_t[:, i * C + mc:i * C + mc + 1],
                        in1=h_t[:, mc, sl], op0=OP.add, op1=OP.add)
                # peel next layer's LN1(tt) (or the final LN on the last
                # layer) here so its scalar/vector chain hides behind the
                # other token-tile's FFN matmuls
                if i + 1 < nb_run:
                    if tt == 0:
                        ln_next = _alloc_ln(pools)
                    g_ap, b_ap, triv = ln_params(2 * (i + 1))
                    _emit_ln_tt(nc, pools, h_t, ones_t, eps2_t, g_ap, b_ap,
                                triv, tt, *ln_next)
                    if tt == 1:
                        xn_next = ln_next[2]
                elif nb_run == NB:
                    # final LN: stats only; the normalize affine is folded
                    # into the logits output: logits = r*(P - s1*cs/E) (+ob)
                    # where P = hb@ow. t1 = P - s1*cs/E is computed as soon
                    # as the stats land so only r (std->recip chain) remains
                    # on the tail.
                    def emit_logits_pre(ltt):
                        sl2 = slice(ltt * 512, ltt * 512 + 512)
                        s1c = lnf_stats[ltt][1]
                        lg = ps1.tile([V, 512], F32, tag="ps1")
                        for c in range(C):
                            nc.tensor.matmul(lg[:], ow_t[:, c, :],
                                             ln_next[0][:, c, sl2],
                                             start=(c == 0),
                                             stop=(c == C - 1))
                        t1 = stats_bf.tile([128, 512], BF16, tag="lgt")
                        nc.vector.scalar_tensor_tensor(
                            out=t1[0:V, :], in0=s1c[0:V, :], scalar=ncs_t[:],
                            in1=lg[:], op0=OP.mult, op1=OP.add)
                        lnf_stats[ltt] = (lnf_stats[ltt][0], t1)
                    if tt == 0:
                        ln_next = _alloc_ln(pools)
                    else:
                        emit_logits_pre(0)
                    r_bf, s1 = _emit_ln_tt(
                        nc, pools, h_t, ones_t, eps2_t, None, None, True,
                        tt, *ln_next, apply=False)
                    s1c = stats.tile([128, 512], F32, tag="stats")
                    nc.scalar.copy(s1c[0:V, :], s1[0:V, :])
                    lnf_stats[tt] = (r_bf, s1c)
                    if tt == 1:
                        emit_logits_pre(1)

        # ---- logits tail: out = t1 * r (+ ob) ----
        if nb_run == NB:
            for tt in range(2):
                sl = slice(tt * 512, tt * 512 + 512)
                r_bf, t1 = lnf_stats[tt]
                lgs = stats.tile([128, 512], F32, tag="stats")
                nc.vector.tensor_tensor(lgs[0:V, :], t1[0:V, :],
                                        r_bf[0:V, :], OP.mult)
                if not ob_trivial:
                    nc.vector.tensor_scalar_add(lgs[0:V, :], lgs[0:V, :],
                                                ob_t[:])
                nc.sync.dma_start(out_d[:, sl], lgs[0:V, :])
        else:
            xnf = _emit_ln(nc, pools, h_t, ones_t, eps2_t, None, None, True)
            for tt in range(2):
                sl = slice(tt * 512, tt * 512 + 512)
                lg = ps1.tile([V, 512], F32, tag="ps1")
                lgs = stats.tile([128, 512], F32, tag="stats")
                for c in range(C):
                    nc.tensor.matmul(lg[:], ow_t[:, c, :], xnf[:, c, sl],
                                     start=(c == 0), stop=(c == C - 1))
                nc.vector.tensor_scalar_add(lgs[0:V, :], lg[:], ob_t[:])
                nc.sync.dma_start(out_d[:, sl], lgs[0:V, :])

    nc.finalize()
    return nc


def prepare_inputs(inputs):
    """Host-side preprocessing: embedding gather, weight layout + bf16 cast.
    Returns (shared_map, per_core_h0_list, ln_trivial, bias_trivial)."""
    f32 = np.float32
    bf16 = ml_dtypes.bfloat16
    x = np.asarray(inputs["x"]).astype(np.int64)
    emb = np.asarray(inputs["emb"], dtype=f32)
    pos = np.asarray(inputs["pos"], dtype=f32)

    positions = np.minimum(np.arange(T), L - 1)
    h0 = emb[x] + pos[positions][None, :, :]      # [B, T, E] fp32

    def to_dev_lhst(mat, kchunks, mcols):
        m = np.ascontiguousarray(mat.astype(bf16))
        return m.reshape(kchunks, 128, mcols).transpose(1, 0, 2).reshape(
            128, kchunks * mcols)

    wq = np.asarray(inputs["wq"], dtype=f32)
    wk = np.asarray(inputs["wk"], dtype=f32)
    wv = np.asarray(inputs["wv"], dtype=f32)
    pw = np.asarray(inputs["proj_w"], dtype=f32)
    f1 = np.asarray(inputs["ff_w1"], dtype=f32)
    f2 = np.asarray(inputs["ff_w2"], dtype=f32)

    wq_dev = np.stack([to_dev_lhst(wq[i].transpose(1, 0, 2).reshape(E, NH * HS),
                                   C, 512) for i in range(NB)])
    wk_dev = np.stack([to_dev_lhst(wk[i].transpose(1, 0, 2).reshape(E, NH * HS),
                                   C, 512) for i in range(NB)])
    wv_dev = np.stack([to_dev_lhst(wv[i].transpose(1, 0, 2).reshape(E, NH * HS),
                                   C, 512) for i in range(NB)])
    # o_t holds [h1|h0] per head-pair chunk (see vt layout in build_program):
    # permute proj_w rows to match
    pw_perm = np.concatenate([
        np.concatenate([np.arange(hp * 128 + 64, hp * 128 + 128),
                        np.arange(hp * 128, hp * 128 + 64)])
        for hp in range(HP)])
    pw_dev = np.stack([to_dev_lhst(pw[i][pw_perm], C, 512)
                       for i in range(NB)])
    f1_dev = np.stack([to_dev_lhst(f1[i], C, FF) for i in range(NB)])
    f2_dev = np.stack([to_dev_lhst(f2[i], CF, 512) for i in range(NB)])

    def vec_dev(v, chunks):
        return np.ascontiguousarray(v.astype(f32).reshape(chunks, 128).T)

    pb_dev = np.concatenate([vec_dev(np.asarray(inputs["proj_b"][i]), C)
                             for i in range(NB)], axis=1)
    fb1_dev = np.concatenate([vec_dev(np.asarray(inputs["ff_b1"][i]), CF)
                              for i in range(NB)], axis=1)
    fb2_dev = np.concatenate([vec_dev(np.asarray(inputs["ff_b2"][i]), C)
                              for i in range(NB)], axis=1)
    # fold the final-LN affine into the logits head:
    # logits = LNstat(h)*g@W/T + b@W/T + ob/T = r*(h@W') - mu*colsum(W') + ob'
    lnf_g_v = np.asarray(inputs["lnf_g"], dtype=f32)
    lnf_b_v = np.asarray(inputs["lnf_b"], dtype=f32)
    out_w_v = np.asarray(inputs["out_w"], dtype=f32)
    ow_eff = (lnf_g_v[:, None] * out_w_v) / TEMP
    ow_dev = to_dev_lhst(ow_eff, C, V)
    ncs_dev = np.ascontiguousarray((-ow_eff.sum(axis=0) / E).reshape(V, 1))
    ob_eff = (np.asarray(inputs["out_b"], dtype=f32)
              + lnf_b_v @ out_w_v) / TEMP
    ob_dev = np.ascontiguousarray(ob_eff.reshape(V, 1))
    ob_trivial = bool(np.all(ob_eff == 0.0))
    tri_dev = np.triu(np.ones((128, 128), dtype=f32)).astype(bf16)

    gs, bs, ln_trivial = [], [], []
    for i in range(NB):
        for nm_g, nm_b in (("ln1_g", "ln1_b"), ("ln2_g", "ln2_b")):
            g = np.asarray(inputs[nm_g][i], dtype=f32)
            b = np.asarray(inputs[nm_b][i], dtype=f32)
            gs.append(vec_dev(g, C))
            bs.append(vec_dev(b, C))
            ln_trivial.append(bool(np.all(g == 1.0) and np.all(b == 0.0)))
    g = np.asarray(inputs["lnf_g"], dtype=f32)
    b = np.asarray(inputs["lnf_b"], dtype=f32)
    gs.append(vec_dev(g, C))
    bs.append(vec_dev(b, C))
    ln_trivial.append(bool(np.all(g == 1.0) and np.all(b == 0.0)))
    lng_dev = np.concatenate(gs, axis=1)
    lnb_dev = np.concatenate(bs, axis=1)

    ln_trivial.append(ob_trivial)   # threaded through to build_program

    shared = {
        "wq": wq_dev, "wk": wk_dev, "wv": wv_dev, "pw": pw_dev,
        "f1": f1_dev, "f2": f2_dev, "pb": pb_dev, "fb1": fb1_dev,
        "fb2": fb2_dev, "ow": ow_dev, "ob": ob_dev, "ncs": ncs_dev,
        "tri": tri_dev, "lng": lng_dev, "lnb": lnb_dev,
    }

    h0_cores = []
    for core in range(NCORES):
        hh = h0[SEQ * core:SEQ * core + SEQ]          # [SEQ, T, E]
        hT = hh.transpose(2, 0, 1).reshape(E, NTOK)   # [E, NTOK]
        h0_cores.append(np.ascontiguousarray(
            hT.reshape(C, 128, NTOK).transpose(1, 0, 2).reshape(
                128, C * NTOK)))
    bias_trivial = []
    for i in range(NB):
        bias_trivial.append(bool(
            np.all(np.asarray(inputs["proj_b"][i]) == 0.0)
            and np.all(np.asarray(inputs["ff_b1"][i]) == 0.0)
            and np.all(np.asarray(inputs["ff_b2"][i]) == 0.0)))
    return shared, h0_cores, ln_trivial, bias_trivial


def assemble_output(core_logits):
    """core_logits: list of [V, NTOK] fp32 -> [B, T, V]."""
    out = np.empty((B, T, V), np.float32)
    for core in range(NCORES):
        lg = core_logits[core].reshape(V, SEQ, T)
        out[SEQ * core:SEQ * core + SEQ] = lg.transpose(1, 2, 0)
    return out


def get_program(ln_trivial, bias_trivial):
    key = (tuple(ln_trivial), tuple(bias_trivial))
    if key not in _PROGRAM_CACHE:
        lt = list(key[0])
        _PROGRAM_CACHE[key] = build_program(
            lt[:2 * NB + 1], list(key[1]), ob_trivial=bool(lt[-1]))
    return _PROGRAM_CACHE[key]


def reset_device():
    """Recover a wedged accelerator (axon session reset). Best-effort."""
    try:
        import ctypes
        import jax
        jax.devices()
        lib = ctypes.CDLL('/opt/axon/libaxon_pjrt.so')
        lib.axon_reset.restype = ctypes.c_int64
        lib.axon_reset()
    except Exception:
        pass


def kernel(**inputs):
    from concourse.bass_utils import run_bass_kernel_spmd
    shared, h0_cores, ln_trivial, bias_trivial = prepare_inputs(inputs)
    nc = get_program(ln_trivial, bias_trivial)
    in_maps = [dict(shared, h0=h0_cores[c]) for c in range(NCORES)]
    try:
        res = run_bass_kernel_spmd(nc, in_maps, core_ids=list(range(NCORES)))
    except Exception:
        # A previous (profiled) session can leave the device wedged; reset
        # the axon session and retry once.
        reset_device()
        res = run_bass_kernel_spmd(nc, in_maps, core_ids=list(range(NCORES)))
    return assemble_output([res.results[c]["logits"] for c in range(NCORES)])

